# revision 1
# baseline (speedup 1.0000x reference)
"""Trainium2 Bass kernel for nn_DecoderLayer_60060822667509.

Data-parallel over the 4096 tokens (512/core on 8 cores). Routing
(host-side argmax on small logits, mirroring the reference's .item()
syncs) is computed from the actual inputs at call time and a
specialized Bass/Tile program is emitted for the selected DAG.

Activations live feature-major on-chip ([128 features, NFC chunks, TOK
tokens]) so matmul outputs feed the next matmul's moving operand with
no transposes. LayerNorm affines, selection softmax weights and node
activation weights are folded into weight matrices host-side; residual
scalars ride along symbolically on each value. Attention (act 0) keys/
values are exchanged between the two cores sharing a batch via an
AllGather pair group.
"""
import numpy as np
import ml_dtypes
from contextlib import ExitStack

import concourse.bass as bass
import concourse.tile as tile
from concourse import mybir
from concourse.bass import ts
from concourse.bass_utils import run_bass_kernel_spmd
from concourse.masks import make_identity

F32 = mybir.dt.float32
BF16 = mybir.dt.bfloat16
AF = mybir.ActivationFunctionType
ALU = mybir.AluOpType

ISIZE = 512
NHEAD = 8
DH = ISIZE // NHEAD  # 64
NNOD = 8
MAXP = 5
TAU = 1.0
EPS = 1e-6
B = 4
SLEN = 1024
NCORE = 8
TOK = (B * SLEN) // NCORE  # 512 tokens per core
NFC = ISIZE // 128  # feature chunks
NTT = TOK // 128    # token tiles


# ---------------------------------------------------------------------------
# Host-side routing (mirrors reference._routing exactly)
# ---------------------------------------------------------------------------

def _qmask(nsrc):
    m = np.zeros((nsrc, 5), bool)
    m[0, :] = True
    return m.reshape(-1)


def _routing(node_p, edge_p):
    node_p = np.asarray(node_p)
    edge_p = np.asarray(edge_p)
    routes, lind = [], 0
    for c in range(NNOD):
        nsrc = min(c + 2, MAXP)
        snode = c - nsrc
        ep = edge_p[:, lind:lind + nsrc, :].reshape(3, -1)
        qm = _qmask(nsrc)
        nact = int(np.argmax(node_p[c]))
        qsel = int(np.argmax(np.where(qm, -np.inf, ep[0])))
        r = dict(lind=lind, nsrc=nsrc, snode=snode, act=nact, q=qsel, k=None,
                 v=None, ktype=None, km=None, vmode=None)
        if nact < 7:
            km = qm if nact > 0 else None
            kl = ep[1] if km is None else np.where(km, -np.inf, ep[1])
            r['k'] = int(np.argmax(kl))
            r['km'] = km
            r['ktype'] = -2 if r['k'] // 5 == 0 else -1
            if nact < 5:
                if nact == 0 and r['ktype'] == -2:
                    r['v'] = int(np.argmax(ep[2][:5]))
                    r['vmode'] = 'first5'
                else:
                    vl = ep[2] if km is None else np.where(km, -np.inf, ep[2])
                    r['v'] = int(np.argmax(vl))
                    r['vmode'] = 'full'
        routes.append(r)
        lind += nsrc
    return routes


def _softmax_np(x):
    x = np.asarray(x, np.float64)
    e = np.exp(x - x.max())
    return e / e.sum()


def _selw_np(logits, mask, sel):
    logits = np.asarray(logits, np.float64)
    if mask is not None:
        logits = np.where(np.asarray(mask), -np.inf, logits)
    return float(_softmax_np(logits / TAU)[sel])


# ---------------------------------------------------------------------------
# TileContext with a walrus-compatible tail drain: this compiler build
# rejects sem waits on SP Drain/NoOp (TPB_CTRL has no wait slots), so
# emit the end-of-kernel waits as standalone wait_ge instructions.
# ---------------------------------------------------------------------------

class FixedTileContext(tile.TileContext):
    def _drain_and_barrier(self, tick_clock, wait_clock):
        nc = self.nc
        clock = list(tick_clock.global_clock)
        for p, sem in sorted(self.sems.allocated().items()):
            c = clock[p]
            if c > 0:
                mult = 16 if sem.name.startswith("DMA") else 1
                nc.sync.wait_ge(sem, c * mult)
        nc.sync.drain()
        nc.all_engine_barrier()
        popped = nc._tile_sem_poison_stack.pop()
        assert popped is self._sem_poison
        nc.clear_and_free_semaphores(list(self.sems.allocated().values()))
        nc.all_engine_barrier()


# ---------------------------------------------------------------------------
# Device-tensor / value abstractions
# ---------------------------------------------------------------------------

class DT:
    """A per-core feature-major tensor: [128 part, NFC, TOK].
    Tiles can be spilled to DRAM and reloaded on demand (DTs are
    write-once, so a spill copy stays valid forever)."""
    def __init__(self, bld):
        self.bld = bld
        self.f32 = None
        self.bf = None
        self.spill = {}

    def _load(self, attr):
        b = self.bld
        dt_ = F32 if attr == "f32" else BF16
        t = b.acq([128, NFC, TOK], dt_)
        b.nc.sync.dma_start(t[:, :, :], self.spill[attr][:, :, :])
        setattr(self, attr, t)
        return t

    def need_bf(self):
        if self.bf is None:
            if "bf" in self.spill:
                return self._load("bf")
            if self.f32 is None and "f32" in self.spill:
                self._load("f32")
            assert self.f32 is not None
            b = self.bld
            self.bf = b.acq([128, NFC, TOK], BF16)
            for fc in range(NFC):
                b.nc.vector.tensor_copy(self.bf[:, fc, :], self.f32[:, fc, :])
        return self.bf

    def need_f32(self):
        if self.f32 is None:
            if "f32" in self.spill:
                return self._load("f32")
            if self.bf is None and "bf" in self.spill:
                self._load("bf")
            assert self.bf is not None
            b = self.bld
            self.f32 = b.acq([128, NFC, TOK], F32)
            for fc in range(NFC):
                b.nc.vector.tensor_copy(self.f32[:, fc, :], self.bf[:, fc, :])
        return self.f32

    def do_spill(self):
        b = self.bld
        for attr in ("f32", "bf"):
            t = getattr(self, attr)
            if t is None:
                continue
            if attr not in self.spill:
                d = b.nc.dram_tensor(
                    b.tag("sp"), [128, NFC, TOK],
                    F32 if attr == "f32" else BF16)
                b.nc.sync.dma_start(d[:, :, :], t[:, :, :])
                self.spill[attr] = d
            b.rel_tile(t)
            setattr(self, attr, None)

    def any(self):
        """Whichever representation exists (no conversion pass); engines
        convert dtypes on read."""
        if self.f32 is not None:
            return self.f32
        if self.bf is not None:
            return self.bf
        if "bf" in self.spill:
            return self._load("bf")
        return self._load("f32")

    def tiles(self):
        return [t for t in (self.f32, self.bf) if t is not None]


class Val:
    """dt scaled by host scalar `mult`; unit=True => per-token zero mean,
    unit variance (LayerNorm output)."""
    def __init__(self, dt, mult=1.0, unit=False):
        self.dt = dt
        self.mult = float(mult)
        self.unit = unit


class Builder:
    def __init__(self, nc, tc, ctx):
        self.nc = nc
        self.tc = tc
        self.uploads = {}
        self.n_tag = 0
        self.act_pool = ctx.enter_context(tc.tile_pool(name="act", bufs=1))
        self.w_pool = ctx.enter_context(tc.tile_pool(name="w", bufs=2))
        self.small_pool = ctx.enter_context(tc.tile_pool(name="small", bufs=1))
        self.ps_pool = ctx.enter_context(
            tc.tile_pool(name="ps", bufs=6, space="PSUM"))
        self.ps_stat = ctx.enter_context(
            tc.tile_pool(name="pstat", bufs=2, space="PSUM"))
        self.ln_cache = {}
        self.live_provider = lambda: set()
        # tile lifetime management
        self.freelist = {}
        self.meta = {}
        self.released = set()
        self.window = []
        # constants
        self.ident_f32 = self.small_pool.tile([128, 128], F32, tag="idf")
        make_identity(nc, self.ident_f32)
        self.ident_bf = self.small_pool.tile([128, 128], BF16, tag="idb")
        make_identity(nc, self.ident_bf)
        self.ones_bf = self.small_pool.tile([128, 1], BF16, tag="ones")
        nc.vector.memset(self.ones_bf, 1.0)
        self.ones_row_f32 = self.small_pool.tile([1, 128], F32, tag="onesr")
        nc.vector.memset(self.ones_row_f32, 1.0)
        self.ones_row_bf = self.small_pool.tile([1, 128], BF16, tag="onesrb")
        nc.vector.memset(self.ones_row_bf, 1.0)
        self.stats_cache = {}

    def tag(self, kind="t"):
        self.n_tag += 1
        return f"{kind}{self.n_tag}"

    # -- recyclable SBUF tiles ----------------------------------------------
    def acq(self, shape, dtype, kind="a"):
        key = (tuple(shape), str(dtype))
        lst = self.freelist.get(key)
        tag = lst.pop() if lst else self.tag(kind)
        t = self.act_pool.tile(list(shape), dtype, tag=tag)
        self.meta[id(t)] = (key, tag)
        self.window.append(t)
        return t

    def rel_tile(self, t):
        if t is None:
            return
        i = id(t)
        if i in self.released or i not in self.meta:
            return
        key, tag = self.meta[i]
        self.freelist.setdefault(key, []).append(tag)
        self.released.add(i)

    def flush(self, keep_vals=(), keep_tiles=()):
        keep = set(self.live_provider())
        for v in keep_vals:
            if v is not None:
                for t in v.dt.tiles():
                    keep.add(id(t))
        for t in keep_tiles:
            if t is not None:
                keep.add(id(t))
        for t in self.window:
            if id(t) not in keep:
                self.rel_tile(t)
        self.window = [t for t in self.window if id(t) in keep]

    def const_col(self, value, parts=128):
        key = (float(value), parts)
        if not hasattr(self, "_cc_cache"):
            self._cc_cache = {}
        if key not in self._cc_cache:
            t = self.small_pool.tile([parts, 1], F32, tag=self.tag("cc"))
            self.nc.vector.memset(t, float(value))
            self._cc_cache[key] = t
        return self._cc_cache[key]

    # -- host->device uploads -----------------------------------------------
    def upload(self, base, arrs, shape, dtype):
        name = f"{base}{len(self.uploads)}"
        if not isinstance(arrs, list):
            arrs = [arrs] * NCORE
        self.uploads[name] = [np.ascontiguousarray(a) for a in arrs]
        return self.nc.declare_dram_parameter(name, list(shape), dtype,
                                              isOutput=False)

    def upload_weight(self, w_np):
        """w_np [512, 512] -> bf16 SBUF tile [128, NFC, 512]."""
        arr = np.ascontiguousarray(
            np.asarray(w_np, np.float32).reshape(NFC, 128, ISIZE)
            .transpose(1, 0, 2)).astype(ml_dtypes.bfloat16)
        hdl = self.upload("w", arr, [128, NFC, ISIZE], BF16)
        t = self.w_pool.tile([128, NFC, ISIZE], BF16, tag="w")
        self.nc.sync.dma_start(t[:, :, :], hdl[:, :, :])
        return t

    def upload_bias(self, b_np):
        """b_np [512] -> SBUF [128, NFC] f32 (per-partition scalars)."""
        arr = np.ascontiguousarray(
            np.asarray(b_np, np.float32).reshape(NFC, 128).transpose(1, 0))
        hdl = self.upload("b", arr, [128, NFC], F32)
        t = self.small_pool.tile([128, NFC], F32, tag=self.tag("bias"))
        self.nc.sync.dma_start(t[:, :], hdl[:, :])
        return t

    # -- emission helpers ----------------------------------------------------
    def load_input_fm(self, hdl):
        """DRAM [TOK, 512] bf16 token-major -> feature-major DT (bf16)."""
        nc = self.nc
        dt = DT(self)
        dt.bf = self.acq([128, NFC, TOK], BF16)
        tok_tiles = []
        for tt in range(NTT):
            t = self.acq([128, ISIZE], BF16)
            nc.sync.dma_start(t[:, :], hdl[ts(tt, 128), :])
            tok_tiles.append(t)
        for fc in range(NFC):
            ps = self.ps_pool.tile([128, TOK], BF16, tag="ps")
            for tt in range(NTT):
                nc.tensor.transpose(ps[:, ts(tt, 128)],
                                    tok_tiles[tt][:, ts(fc, 128)],
                                    self.ident_bf)
            nc.scalar.activation(dt.bf[:, fc, :], ps[:, :], AF.Identity)
        return Val(dt, 1.0, False)

    def mm_psums(self, parts):
        """Matmuls accumulating into NFC psum tiles [128, TOK]; returns them.
        parts: list of (Val, W_np[512,512]); Val.mult folded into W."""
        nc = self.nc
        wts = [self.upload_weight(np.asarray(w, np.float64) * v.mult)
               for v, w in parts]
        rhs = [v.dt.need_bf() for v, _ in parts]
        psums = []
        for mc in range(NFC):
            ps = self.ps_pool.tile([128, TOK], F32, tag="ps")
            first = True
            for wi, (wt, r) in enumerate(zip(wts, rhs)):
                for kc in range(NFC):
                    nc.tensor.matmul(ps[:, :], wt[:, kc, ts(mc, 128)],
                                     r[:, kc, :], start=first,
                                     stop=(wi == len(wts) - 1 and
                                           kc == NFC - 1))
                    first = False
            psums.append(ps)
        return psums

    def matmul_fm(self, parts, bias_np=None, epi="identity", epi_scale=1.0,
                  out_f32=True, out_bf=False):
        """epi( sum_i (mult_i*x_i) @ W_i + bias ) -> Val(mult=1).
        epi in {identity, relu, gelu}; epi_scale pre-scales inside relu."""
        nc = self.nc
        psums = self.mm_psums(parts)
        bias_t = None
        if bias_np is not None and np.any(bias_np):
            bias_t = self.upload_bias(
                np.asarray(bias_np, np.float64) *
                (epi_scale if epi == "relu" else 1.0))
        dt = DT(self)
        if out_f32:
            dt.f32 = self.acq([128, NFC, TOK], F32)
        if out_bf:
            dt.bf = self.acq([128, NFC, TOK], BF16)
        func = {"identity": AF.Identity, "relu": AF.Relu,
                "gelu": AF.Gelu_apprx_tanh}[epi]
        for mc, ps in enumerate(psums):
            bias_ap = bias_t[:, mc:mc + 1] if bias_t is not None else 0.0
            scale = epi_scale if epi == "relu" else 1.0
            tgt = dt.f32 if dt.f32 is not None else dt.bf
            nc.scalar.activation(tgt[:, mc, :], ps[:, :], func,
                                 bias=bias_ap, scale=scale)
            if dt.f32 is not None and dt.bf is not None:
                nc.vector.tensor_copy(dt.bf[:, mc, :], dt.f32[:, mc, :])
        return Val(dt, 1.0, False)

    def act_pass(self, val, func, scale=1.0):
        """Elementwise ACT func(scale*mult*x) -> Val(mult=1), bf16."""
        nc = self.nc
        src = val.dt.any()
        dt = DT(self)
        dt.bf = self.acq([128, NFC, TOK], BF16)
        for fc in range(NFC):
            nc.scalar.activation(dt.bf[:, fc, :], src[:, fc, :], func,
                                 scale=float(scale * val.mult))
        return Val(dt, 1.0, False)

    def axpy(self, a, b, out_bf=False):
        """a.mult*a + b.mult*b (one DVE pass)."""
        nc = self.nc
        if abs(a.mult) > abs(b.mult):
            a, b = b, a
        dt = DT(self)
        t = self.acq([128, NFC, TOK], BF16 if out_bf else F32)
        if out_bf:
            dt.bf = t
        else:
            dt.f32 = t
        aa, bb = a.dt.any(), b.dt.any()
        for fc in range(NFC):
            nc.vector.scalar_tensor_tensor(
                t[:, fc, :], aa[:, fc, :], float(a.mult / b.mult),
                bb[:, fc, :], op0=ALU.mult, op1=ALU.add)
        return Val(dt, b.mult, False)

    def mul_vals(self, a, b, extra=1.0):
        nc = self.nc
        dt = DT(self)
        dt.f32 = self.acq([128, NFC, TOK], F32)
        aa, bb = a.dt.any(), b.dt.any()
        for fc in range(NFC):
            nc.vector.tensor_mul(dt.f32[:, fc, :], aa[:, fc, :],
                                 bb[:, fc, :])
        return Val(dt, a.mult * b.mult * extra, False)

    def add_psum_resid(self, resid, resid_scale, psums):
        """resid.t * resid_scale + psum (per-chunk fused passes)."""
        nc = self.nc
        dt = DT(self)
        dt.f32 = self.acq([128, NFC, TOK], F32)
        rt = resid.dt.any()
        for mc, ps in enumerate(psums):
            nc.vector.scalar_tensor_tensor(
                dt.f32[:, mc, :], rt[:, mc, :], float(resid_scale),
                ps[:, :], op0=ALU.mult, op1=ALU.add)
        return Val(dt, 1.0, False)

    def ln_stats(self, val):
        """Per-token LN statistics of a feature-major value, for fused-LN
        matmuls: returns (m_bf [1,TOK] bf16, rb_sb [128,TOK] bf16 broadcast
        of rstd). Cached per underlying tensor."""
        key = (id(val.dt), round(float(val.mult), 12))
        c = self.stats_cache.get(key)
        if c is not None:
            return c[1], c[2]
        nc = self.nc
        xbf = val.dt.need_bf()
        x2 = self.acq([128, NFC, TOK], BF16)
        for fc in range(NFC):
            nc.vector.tensor_mul(x2[:, fc, :], xbf[:, fc, :], xbf[:, fc, :])
        m_ps = self.ps_stat.tile([1, TOK], F32, tag="st")
        s2_ps = self.ps_stat.tile([1, TOK], F32, tag="st")
        for kc in range(NFC):
            nc.tensor.matmul(m_ps[:, :], self.ones_bf[:, :], xbf[:, kc, :],
                             start=(kc == 0), stop=(kc == NFC - 1))
        for kc in range(NFC):
            nc.tensor.matmul(s2_ps[:, :], self.ones_bf[:, :], x2[:, kc, :],
                             start=(kc == 0), stop=(kc == NFC - 1))
        sm = self.acq([1, 3 * TOK], F32)
        s0, s1, s2 = (sm[:, ts(i, TOK)] for i in range(3))
        nc.vector.tensor_scalar_mul(s0, m_ps[:, :], 1.0 / ISIZE)   # mean
        nc.vector.scalar_tensor_tensor(s2, s0, -1.0, s0,
                                       op0=ALU.mult, op1=ALU.mult)
        nc.vector.scalar_tensor_tensor(s1, s2_ps[:, :], 1.0 / ISIZE, s2,
                                       op0=ALU.mult, op1=ALU.add)   # var
        epsp = EPS / (val.mult * val.mult)
        nc.scalar.activation(s2, s1, AF.Ln, bias=self.const_col(epsp, 1))
        nc.scalar.activation(s1, s2, AF.Exp, scale=-0.5)            # rstd
        m_bf = self.acq([1, TOK], BF16)
        r_bf = self.acq([1, TOK], BF16)
        nc.vector.tensor_copy(m_bf[:, :], s0)
        nc.vector.tensor_copy(r_bf[:, :], s1)
        rb_ps = self.ps_stat.tile([128, TOK], F32, tag="st")
        nc.tensor.matmul(rb_ps[:, :], self.ones_row_bf[:, :], r_bf[:, :],
                         start=True, stop=True)
        rb_sb = self.acq([128, TOK], BF16)
        nc.scalar.activation(rb_sb[:, :], rb_ps[:, :], AF.Identity)
        self.rel_tile(x2)
        self.rel_tile(sm)
        self.rel_tile(r_bf)
        self.stats_cache[key] = (val.mult, m_bf, rb_sb)
        return m_bf, rb_sb

    def matmul_fm_ln(self, val, w_eff, bias_np=None, out_f32=False,
                     out_bf=True):
        """LNraw(val) @ w_eff + bias, with the matmuls running on the RAW
        activations: mean is subtracted inside PSUM via a K=1 matmul with
        the column sums of w_eff, and rstd is applied in the PSUM->SBUF
        epilogue (both commute with the contraction)."""
        nc = self.nc
        m_bf, rb_sb = self.ln_stats(val)
        wbf = np.asarray(w_eff, np.float32).astype(ml_dtypes.bfloat16)
        wt = self.upload_weight(wbf)
        wcs = np.ascontiguousarray(
            -wbf.astype(np.float32).sum(axis=0)[None, :]
        ).astype(ml_dtypes.bfloat16)
        hw = self.upload("wc", wcs, [1, ISIZE], BF16)
        wcs_t = self.acq([1, ISIZE], BF16)
        nc.gpsimd.dma_start(wcs_t[:, :], hw[:, :])
        xbf = val.dt.need_bf()
        dt = DT(self)
        if out_bf:
            dt.bf = self.acq([128, NFC, TOK], BF16)
        if out_f32:
            dt.f32 = self.acq([128, NFC, TOK], F32)
        bias_t = self.upload_bias(bias_np) \
            if bias_np is not None and np.any(bias_np) else None
        for mc in range(NFC):
            ps = self.ps_pool.tile([128, TOK], F32, tag="ps")
            for kc in range(NFC):
                nc.tensor.matmul(ps[:, :], wt[:, kc, ts(mc, 128)],
                                 xbf[:, kc, :], start=(kc == 0), stop=False)
            nc.tensor.matmul(ps[:, :], wcs_t[0:1, ts(mc, 128)], m_bf[:, :],
                             start=False, stop=True)
            tgt = dt.bf if dt.bf is not None else dt.f32
            nc.vector.scalar_tensor_tensor(
                tgt[:, mc, :], ps[:, :], 1.0, rb_sb[:, :],
                op0=ALU.mult, op1=ALU.mult)
            if dt.bf is not None and dt.f32 is not None:
                nc.vector.tensor_copy(dt.f32[:, mc, :], dt.bf[:, mc, :])
            if bias_t is not None:
                for t in dt.tiles():
                    nc.scalar.activation(t[:, mc, :], t[:, mc, :],
                                         AF.Identity,
                                         bias=bias_t[:, mc:mc + 1])
        self.rel_tile(wcs_t)
        return Val(dt, 1.0, False)

    def ln_fm(self, val, out_f32=False, out_bf=True):
        """Feature-major LNraw; scale-invariant up to eps (folded exactly
        into eps'). Unit-LN input collapses to a host scalar."""
        if val.unit:
            kappa = 1.0 / np.sqrt(1.0 + EPS / (val.mult * val.mult))
            return Val(val.dt, kappa, True)
        key = id(val.dt)
        if key in self.ln_cache:
            return self.ln_cache[key][1]
        nc = self.nc
        xs = val.dt.any()
        xbf = val.dt.need_bf()
        x2 = self.acq([128, NFC, TOK], BF16)
        nc.vector.tensor_mul(x2[:, :, :], xs[:, :, :], xs[:, :, :])
        m_ps = self.ps_stat.tile([1, TOK], F32, tag="st")
        s2_ps = self.ps_stat.tile([1, TOK], F32, tag="st")
        for kc in range(NFC):
            nc.tensor.matmul(m_ps[:, :], self.ones_bf[:, :], xbf[:, kc, :],
                             start=(kc == 0), stop=(kc == NFC - 1))
        for kc in range(NFC):
            nc.tensor.matmul(s2_ps[:, :], self.ones_bf[:, :], x2[:, kc, :],
                             start=(kc == 0), stop=(kc == NFC - 1))
        sm = self.acq([1, 3 * TOK], F32)
        s0, s1, s2 = (sm[:, ts(i, TOK)] for i in range(3))
        nc.vector.tensor_scalar_mul(s0, m_ps[:, :], 1.0 / ISIZE)   # mean
        nc.vector.tensor_scalar_mul(s1, s2_ps[:, :], 1.0 / ISIZE)  # E[x^2]
        nc.vector.scalar_tensor_tensor(s2, s0, -1.0, s0,
                                       op0=ALU.mult, op1=ALU.mult)  # -mean^2
        nc.vector.tensor_add(s1, s1, s2)                            # var
        epsp = EPS / (val.mult * val.mult)
        nc.scalar.activation(s2, s1, AF.Ln, bias=self.const_col(epsp, 1))
        nc.scalar.activation(s1, s2, AF.Exp, scale=-0.5)            # rstd
        nc.vector.tensor_mul(s2, s0, s1)                            # mean*rstd
        rstd, mr = s1, s2
        rb_ps = self.ps_stat.tile([128, TOK], F32, tag="st")
        mrb_ps = self.ps_stat.tile([128, TOK], F32, tag="st")
        nc.tensor.matmul(rb_ps[:, :], self.ones_row_f32[:, :], rstd,
                         start=True, stop=True)
        nc.tensor.matmul(mrb_ps[:, :], self.ones_row_f32[:, :], mr,
                         start=True, stop=True)
        rb = self.acq([128, TOK], BF16)
        mrb = self.acq([128, TOK], BF16)
        nc.scalar.activation(rb[:, :], rb_ps[:, :], AF.Identity)
        nc.scalar.activation(mrb[:, :], mrb_ps[:, :], AF.Identity)
        dt = DT(self)
        u = self.acq([128, NFC, TOK], BF16)
        for fc in range(NFC):
            nc.vector.tensor_mul(u[:, fc, :], xs[:, fc, :], rb[:, :])
        targets = []
        if out_bf:
            dt.bf = self.acq([128, NFC, TOK], BF16)
            targets.append(dt.bf)
        if out_f32:
            dt.f32 = self.acq([128, NFC, TOK], F32)
            targets.append(dt.f32)
        for t in targets:
            for fc in range(NFC):
                nc.vector.scalar_tensor_tensor(
                    t[:, fc, :], u[:, fc, :], 1.0, mrb[:, :],
                    op0=ALU.mult, op1=ALU.subtract)
        out = Val(dt, 1.0, True)
        self.ln_cache[key] = (val.dt, out)
        return out

    # -- multi-head attention (act 0) ---------------------------------------
    def emit_mha(self, qv, kv, vv, nW, nb, ng, nbe, aw, core_mask_arrs):
        nc = self.nc
        mid = self.tag("mha")
        w0 = np.asarray(ng, np.float64)[:, None] * np.asarray(nW[0], np.float64)
        b0 = np.asarray(nbe, np.float64) @ np.asarray(nW[0], np.float64) \
            + np.asarray(nb[0], np.float64)
        if qv.unit:
            qn = self.ln_fm(qv)
            qh = self.matmul_fm([(qn, w0)], bias_np=b0, out_f32=False,
                                out_bf=True)
        else:
            qh = self.matmul_fm_ln(qv, w0, bias_np=b0, out_f32=False,
                                   out_bf=True)
        kh = self.matmul_fm([(kv, np.asarray(nW[1], np.float64))],
                            bias_np=np.asarray(nb[1], np.float64),
                            out_f32=False, out_bf=True)
        # vh token-major [128 tok, (h, dh)] with a trailing ones column
        w2t = self.upload_weight(np.asarray(nW[2], np.float64) * vv.mult)
        vbf = vv.dt.need_bf()
        b2 = np.asarray(nb[2], np.float64)
        b2_row = None
        if np.any(b2):
            hb = self.upload("vb", b2.astype(np.float32)[None, :],
                             [1, ISIZE], F32)
            b2_row = self.small_pool.tile([1, ISIZE], F32, tag=self.tag("vb"))
            nc.sync.dma_start(b2_row[:, :], hb[:, :])
        vht = self.acq([128, NTT, NHEAD, DH + 1], BF16)
        for tt in range(NTT):
            ps = self.ps_pool.tile([128, ISIZE], F32, tag="ps")
            for kc in range(NFC):
                nc.tensor.matmul(ps[:, :], vbf[:, kc, ts(tt, 128)],
                                 w2t[:, kc, :], start=(kc == 0),
                                 stop=(kc == NFC - 1 and b2_row is None))
            if b2_row is not None:
                nc.tensor.matmul(ps[:, :], self.ones_row_f32[:, :],
                                 b2_row[:, :], start=False, stop=True)
            nc.scalar.activation(
                vht[:, tt, :, 0:DH],
                ps[:, :].rearrange("p (h d) -> p h d", h=NHEAD),
                AF.Identity)
        nc.vector.memset(vht[:, :, :, DH], 1.0)
        # pairwise AllGather of kh (feature-major) and vht (token-major)
        kh_loc = nc.dram_tensor(f"khl{mid}", [128, NFC, TOK], BF16)
        vh_loc = nc.dram_tensor(f"vhl{mid}", [128, NTT, NHEAD, DH + 1], BF16)
        kh_g = nc.dram_tensor(f"khg{mid}", [2, 128, NFC, TOK], BF16)
        vh_g = nc.dram_tensor(f"vhg{mid}", [2, 128, NTT, NHEAD, DH + 1],
                              BF16)
        nc.sync.dma_start(kh_loc[:, :, :], kh.dt.bf[:, :, :])
        nc.sync.dma_start(vh_loc[:, :, :, :], vht[:, :, :, :])
        groups = [[0, 1], [2, 3], [4, 5], [6, 7]]
        nc.gpsimd.collective_compute(
            "AllGather", ALU.bypass, replica_groups=groups,
            ins=[kh_loc[:, :, :]], outs=[kh_g[:, :, :, :]])
        nc.gpsimd.collective_compute(
            "AllGather", ALU.bypass, replica_groups=groups,
            ins=[vh_loc[:, :, :, :]], outs=[vh_g[:, :, :, :, :]])
        khg = self.acq([128, 2, NFC, TOK], BF16)
        vhg = self.acq([128, 2, NTT, NHEAD, DH + 1], BF16)
        for r in range(2):
            nc.sync.dma_start(khg[:, r, :, :], kh_g[r, :, :, :])
            nc.sync.dma_start(vhg[:, r, :, :, :], vh_g[r, :, :, :, :])
        self.flush(keep_vals=[qv, kv, vv, qh], keep_tiles=[khg, vhg])
        maskb = None
        if core_mask_arrs is not None:
            hb = self.upload("mb", core_mask_arrs, [128, 2 * NTT], F32)
            maskb = self.small_pool.tile([128, 2 * NTT], F32,
                                         tag=self.tag("mb"))
            nc.sync.dma_start(maskb[:, :], hb[:, :])
        qhbf = qh.dt.bf
        oTn = DT(self)
        oTn.bf = self.acq([128, NFC, TOK], BF16)
        scale = 1.0 / float(np.sqrt(DH))
        for h in range(NHEAD):
            po = DH * (h % 2)
            fc = h // 2
            att = self.ps_stat.tile([DH + 1, TOK], F32, tag="st")
            for kc8 in range(2 * NTT):
                r, tl = kc8 // NTT, kc8 % NTT
                sT = self.ps_pool.tile([128, TOK], F32, tag="ps")
                nc.tensor.matmul(sT[:, :],
                                 khg[po:po + DH, r, fc, ts(tl, 128)],
                                 qhbf[po:po + DH, fc, :],
                                 start=True, stop=True)
                bias_ap = maskb[:, kc8:kc8 + 1] if maskb is not None else 0.0
                exp_sb = self.acq([128, TOK], BF16)
                nc.scalar.activation(exp_sb[:, :], sT[:, :], AF.Exp,
                                     bias=bias_ap, scale=scale)
                nc.tensor.matmul(att[:, :],
                                 vhg[:, r, tl, h, :],
                                 exp_sb[:, :], start=(kc8 == 0),
                                 stop=(kc8 == 2 * NTT - 1))
                self.rel_tile(exp_sb)
            # normalize: recip(rowsum) broadcast over the head's partitions
            rs_sb = self.acq([1, TOK], F32)
            nc.scalar.activation(rs_sb[:, :], att[DH:DH + 1, :], AF.Ln)
            nc.scalar.activation(rs_sb[:, :], rs_sb[:, :], AF.Exp, scale=-1.0)
            rb_ps = self.ps_stat.tile([DH, TOK], F32, tag="st")
            nc.tensor.matmul(rb_ps[:, :], self.ones_row_f32[:, 0:DH],
                             rs_sb[:, :], start=True, stop=True)
            rb_sb = self.acq([128, TOK], F32)
            nc.scalar.activation(rb_sb[0:DH, :], rb_ps[:, :], AF.Identity)
            nc.vector.tensor_mul(oTn.bf[po:po + DH, fc, :], att[0:DH, :],
                                 rb_sb[0:DH, :])
            self.rel_tile(rs_sb)
            self.rel_tile(rb_sb)
        self.flush(keep_vals=[qv], keep_tiles=list(oTn.tiles()))
        b3 = np.asarray(nb[3], np.float64)
        w3 = aw * np.asarray(nW[3], np.float64)
        if np.any(b3):
            pr = self.matmul_fm([(Val(oTn, 1.0), w3)], bias_np=aw * b3,
                                out_f32=True)
            return self.axpy(Val(qv.dt, qv.mult * aw, False),
                             Val(pr.dt, 1.0, False))
        psums = self.mm_psums([(Val(oTn, 1.0), w3)])
        return self.add_psum_resid(qv, aw * qv.mult, psums)




# ---------------------------------------------------------------------------
# Walrus-compat post-pass: this compiler build supports at most one sync
# wait on most engine instructions (none on SP control ops). Hoist excess
# waits onto standalone InstEventSemaphore instructions inserted before.
# ---------------------------------------------------------------------------

_NO_HOIST = ("InstEventSemaphore", "InstAllEngineBarrier",
             "InstCollectiveCompute")


def _hoist_excess_waits(nc):
    n = 0
    for f in nc.m.functions:
        for bb in f.blocks:
            out = []
            changed = False
            for inst in bb.instructions:
                tname = type(inst).__name__
                si = inst.sync_info
                if si is not None and tname not in _NO_HOIST:
                    waits = list(si.on_wait)
                    limit = 0 if tname in ("InstDrain", "InstNoOp") else 1
                    if len(waits) > limit:
                        for w in waits[:len(waits) - limit]:
                            n += 1
                            ni = mybir.InstEventSemaphore(
                                name=f"I-hoist{n}", ins=[], outs=[])
                            ni.engine = inst.engine
                            ni.sync_info = mybir.SyncInfo(on_wait=[w],
                                                          on_update=[])
                            out.append(ni)
                        si.on_wait = waits[len(waits) - limit:]
                        changed = True
                out.append(inst)
            if changed:
                bb.instructions = out
    return n


# ---------------------------------------------------------------------------
# Graph emission
# ---------------------------------------------------------------------------

def _emit_graph(bld, np_in, routes, core_mask_bias):
    nc = bld.nc
    eW = np.asarray(np_in['edge_W'], np.float64)
    eb = np.asarray(np_in['edge_b'], np.float64)
    eg = np.asarray(np_in['edge_g'], np.float64)
    ebe = np.asarray(np_in['edge_beta'], np.float64)
    nW = np.asarray(np_in['node_W'], np.float64)
    nb = np.asarray(np_in['node_b'], np.float64)
    ng = np.asarray(np_in['node_g'], np.float64)
    nbe = np.asarray(np_in['node_beta'], np.float64)
    node_p = np.asarray(np_in['node_p'], np.float64)
    edge_p = np.asarray(np_in['edge_p'], np.float64)

    # source lifetimes
    last_use = {}
    use_nodes = {}
    used_src = set()
    for c, r in enumerate(routes):
        for sel in (r['q'], r['k'], r['v']):
            if sel is None:
                continue
            se = sel // 5
            src = -2 if se == 0 else r['snode'] + se
            used_src.add(src)
            last_use[src] = c
            use_nodes.setdefault(src, []).append(c)
    for i in range(NNOD):
        if i not in use_nodes:
            use_nodes[i] = [NNOD]  # survives to the final sum

    # sources that later feed an LN'd edge (ops 0/1/2) want their LN
    # statistics computed as soon as they exist, so fused-LN consumers
    # never stall on the stats chain.
    needs_stats = set()
    for r in routes:
        for sel in (r['q'], r['k'], r['v']):
            if sel is None:
                continue
            se, op = sel // 5, sel % 5
            if op <= 2:
                needs_stats.add(-2 if se == 0 else r['snode'] + se)

    outs = {}
    for nm, idx in (('inpute', -2), ('inputo', -1)):
        if idx in used_src:
            hdl = bld.upload(
                nm,
                [np.ascontiguousarray(
                    np.asarray(np_in[nm]).reshape(-1, ISIZE)
                    [i * TOK:(i + 1) * TOK].astype(ml_dtypes.bfloat16))
                 for i in range(NCORE)],
                [TOK, ISIZE], BF16)
            outs[idx] = bld.load_input_fm(hdl)
            if idx in needs_stats:
                bld.ln_stats(outs[idx])

    edge_cache = {}
    processed = set()

    def edge_value(r, sel, which):
        se, op = sel // 5, sel % 5
        inn = -2 if se == 0 else r['snode'] + se
        processed.add(inn)
        e = r['lind'] + se
        lind, nsrc = r['lind'], r['nsrc']
        ep = edge_p[:, lind:lind + nsrc, :].reshape(3, -1)
        logits = ep[{'q': 0, 'k': 1, 'v': 2}[which]]
        first5 = (which == 'v' and r['vmode'] == 'first5')
        if first5:
            logits = logits[:5]
        mask = _qmask(nsrc) if which == 'q' else r['km']
        if first5:
            mask = None
        s = _selw_np(logits, mask, sel)
        src = outs[inn]
        if op == 4:
            return Val(src.dt, src.mult * s, src.unit)
        if op == 3:
            key = ('p', e)
            if key not in edge_cache:
                edge_cache[key] = bld.matmul_fm(
                    [(src, eW[e])],
                    bias_np=eb[e] if np.any(eb[e]) else None,
                    out_f32=False, out_bf=True)
            return Val(edge_cache[key].dt, s, False)
        key = ('h', e)
        if key not in edge_cache:
            wp = eg[e][:, None] * eW[e]
            bp = ebe[e] @ eW[e] + eb[e]
            if src.unit:
                lnv = bld.ln_fm(src)
                edge_cache[key] = bld.matmul_fm(
                    [(lnv, wp)], bias_np=bp if np.any(bp) else None,
                    out_f32=False, out_bf=True)
            else:
                edge_cache[key] = bld.matmul_fm_ln(
                    src, wp, bias_np=bp if np.any(bp) else None,
                    out_f32=False, out_bf=True)
        h = edge_cache[key]
        if op == 2:
            return Val(h.dt, s, False)
        fkey = ('relu' if op == 0 else 'gelu', e)
        if fkey not in edge_cache:
            edge_cache[fkey] = bld.act_pass(
                h, AF.Relu if op == 0 else AF.Gelu_apprx_tanh)
        return Val(edge_cache[fkey].dt, s, False)

    def affine_node(ln_val, c, aw):
        g, bta = ng[c], nbe[c]
        if np.all(g == 1.0) and not np.any(bta):
            return Val(ln_val.dt, ln_val.mult * aw, True)
        sc = bld.upload_bias(aw * ln_val.mult * g)
        bi = bld.upload_bias(aw * bta)
        dt = DT(bld)
        dt.bf = bld.acq([128, NFC, TOK], BF16)
        src = ln_val.dt.any()
        for fc in range(NFC):
            nc.scalar.activation(dt.bf[:, fc, :], src[:, fc, :], AF.Identity,
                                 scale=sc[:, fc:fc + 1], bias=bi[:, fc:fc + 1])
        return Val(dt, 1.0, False)

    def reachable_ids():
        s = set()
        vals = list(outs.values()) + list(edge_cache.values()) + \
            [lv for _, lv in bld.ln_cache.values()]
        for v in vals:
            for t in v.dt.tiles():
                s.add(id(t))
        for _, m_bf, rb_sb in bld.stats_cache.values():
            s.add(id(m_bf))
            s.add(id(rb_sb))
        return s

    bld.live_provider = reachable_ids
    flush = bld.flush

    for c, r in enumerate(routes):
        act = r['act']
        aw = float(_softmax_np(node_p[c] / TAU)[act])
        qv = edge_value(r, r['q'], 'q')
        flush([qv])
        kv = edge_value(r, r['k'], 'k') if r['k'] is not None else None
        flush([qv, kv])
        vv = edge_value(r, r['v'], 'v') if r['v'] is not None else None
        flush([qv, kv, vv])

        if act == 0:
            mask_nm = 'tgt_pad_mask' if r['ktype'] == -1 else 'src_pad_mask'
            outs[c] = bld.emit_mha(
                qv, kv, vv, nW[c], nb[c], ng[c], nbe[c], aw,
                core_mask_bias(np.asarray(np_in[mask_nm])))
        elif act == 1:
            g = bld.matmul_fm([(qv, nW[c, 0])],
                              bias_np=nb[c, 0] if np.any(nb[c, 0]) else None,
                              epi="gelu", out_f32=False, out_bf=True)
            kk = bld.matmul_fm([(kv, nW[c, 1])],
                               bias_np=nb[c, 1] if np.any(nb[c, 1]) else None,
                               out_f32=False, out_bf=True)
            p = bld.mul_vals(g, kk)
            if np.any(nb[c, 3]):
                pr = bld.matmul_fm([(p, aw * nW[c, 3])], bias_np=aw * nb[c, 3],
                                   out_f32=True)
                outs[c] = bld.axpy(Val(qv.dt, qv.mult * aw, False),
                                   Val(pr.dt, 1.0, False))
            else:
                ps = bld.mm_psums([(p, aw * nW[c, 3])])
                outs[c] = bld.add_psum_resid(qv, aw * qv.mult, ps)
        elif act == 2:
            s2 = bld.axpy(bld.axpy(qv, kv, out_bf=True), vv, out_bf=True)
            ln = bld.ln_fm(s2, out_f32=False, out_bf=True)
            outs[c] = affine_node(ln, c, aw)
        elif act == 3:
            inner = bld.matmul_fm([(qv, nW[c, 0]), (kv, nW[c, 1]),
                                   (vv, nW[c, 2])], epi="relu",
                                  out_f32=False, out_bf=True)
            if np.any(nb[c, 3]):
                pr = bld.matmul_fm([(inner, aw * nW[c, 3])],
                                   bias_np=aw * nb[c, 3], out_f32=True)
                outs[c] = bld.axpy(Val(qv.dt, qv.mult * aw, False),
                                   Val(pr.dt, 1.0, False))
            else:
                ps = bld.mm_psums([(inner, aw * nW[c, 3])])
                outs[c] = bld.add_psum_resid(qv, aw * qv.mult, ps)
        elif act == 4:
            sg = bld.act_pass(kv, AF.Sigmoid)
            p = bld.mul_vals(qv, sg)
            outs[c] = bld.axpy(Val(p.dt, p.mult * aw, False),
                               Val(vv.dt, vv.mult * aw, vv.unit))
        elif act == 5:
            kk = bld.matmul_fm([(kv, nW[c, 1])],
                               bias_np=nb[c, 1] if np.any(nb[c, 1]) else None,
                               epi="gelu", out_f32=False, out_bf=True)
            outs[c] = bld.axpy(Val(kk.dt, aw, False),
                               Val(qv.dt, qv.mult * aw, qv.unit))
        elif act == 6:
            outs[c] = bld.axpy(Val(qv.dt, qv.mult * aw, qv.unit),
                               Val(kv.dt, kv.mult * aw, kv.unit))
        else:
            ln = bld.ln_fm(qv, out_f32=False, out_bf=True)
            outs[c] = affine_node(ln, c, aw)

        if c in needs_stats and not outs[c].unit:
            bld.ln_stats(outs[c])

        # ---- lifetime bookkeeping ----
        dead_tiles = []
        for s_idx in [s for s, lc in last_use.items() if lc == c]:
            v = outs.pop(s_idx, None)
            if v is not None:
                dead_tiles += v.dt.tiles()
        # prune LN/stats cache entries whose source is no longer alive
        alive_dts = {id(v.dt) for v in outs.values()}
        for key in [k for k in bld.ln_cache if k not in alive_dts]:
            _, lv = bld.ln_cache.pop(key)
            dead_tiles += lv.dt.tiles()
        for key in [k for k in bld.stats_cache if k[0] not in alive_dts]:
            _, m_bf, rb_sb = bld.stats_cache.pop(key)
            dead_tiles += [m_bf, rb_sb]
        edge_cache.clear()
        keep = reachable_ids()
        for t in bld.window + dead_tiles:
            if id(t) not in keep:
                bld.rel_tile(t)
        bld.window = []
        # spill node outputs whose next use is at least two nodes away
        for s_idx, v in list(outs.items()):
            nxt = min((u for u in use_nodes.get(s_idx, [NNOD]) if u > c),
                      default=NNOD)
            if nxt > c + 1:
                lv = bld.ln_cache.get(id(v.dt))
                if lv is not None:
                    lv[1].dt.do_spill()
                v.dt.do_spill()

    rem = [outs[i] for i in range(NNOD) if i not in processed]
    acc = rem[0]
    for t in rem[1:]:
        acc = bld.axpy(acc, t)
    return acc


def _emit_final(bld, acc, out_hdl, out_g, out_beta):
    """Transpose to token-major, final LNraw (+ optional affine), DMA out."""
    nc = bld.nc
    x = acc.dt.need_f32()
    epsp = EPS / (acc.mult * acc.mult)
    need_aff = not (np.all(out_g == 1.0) and not np.any(out_beta))
    if need_aff:
        gh = bld.upload("og", np.tile(np.asarray(out_g, np.float32),
                                      (128, 1)), [128, ISIZE], F32)
        bh = bld.upload("ob", np.tile(np.asarray(out_beta, np.float32),
                                      (128, 1)), [128, ISIZE], F32)
        gt = bld.acq([128, ISIZE], F32)
        bt = bld.acq([128, ISIZE], F32)
        nc.sync.dma_start(gt[:, :], gh[:, :])
        nc.sync.dma_start(bt[:, :], bh[:, :])
    eps_col = bld.const_col(epsp, 128)
    for tt in range(NTT):
        ps = bld.ps_pool.tile([128, ISIZE], F32, tag="ps")
        for fc in range(NFC):
            nc.tensor.transpose(ps[:, ts(fc, 128)], x[:, fc, ts(tt, 128)],
                                bld.ident_f32)
        sm = bld.acq([128, 12], F32)
        stats, mv, rstd = sm[:, 0:6], sm[:, 6:8], sm[:, 8:9]
        nc.vector.bn_stats(stats, ps[:, :])
        nc.vector.bn_aggr(mv, stats)
        nc.scalar.activation(rstd, mv[:, 1:2], AF.Ln, bias=eps_col)
        nc.scalar.activation(rstd, rstd, AF.Exp, scale=-0.5)
        ot = bld.acq([128, ISIZE], F32)
        nc.vector.tensor_scalar(ot[:, :], ps[:, :], mv[:, 0:1], rstd,
                                op0=ALU.subtract, op1=ALU.mult)
        if need_aff:
            nc.vector.tensor_mul(ot[:, :], ot[:, :], gt[:, :])
            nc.vector.tensor_add(ot[:, :], ot[:, :], bt[:, :])
        nc.sync.dma_start(out_hdl[ts(tt, 128), :], ot[:, :])
        bld.rel_tile(sm)
        bld.rel_tile(ot)


def _build_and_run(inputs, trace=False, **run_kwargs):
    np_in = {k: np.asarray(v) for k, v in inputs.items()}
    routes = _routing(np_in['node_p'], np_in['edge_p'])

    def core_mask_bias(mask_np):
        if not np.any(mask_np):
            return None
        arrs = []
        for core in range(NCORE):
            vec = np.asarray(mask_np[core // 2, 0, :], bool)
            mb = np.zeros((128, 2 * NTT), np.float32)
            for kc8 in range(2 * NTT):
                base = (kc8 // NTT) * TOK + (kc8 % NTT) * 128
                mb[:, kc8] = np.where(vec[base:base + 128], -1e9, 0.0)
            arrs.append(mb)
        return arrs

    nc = bass.Bass(num_devices=NCORE)
    out_hdl = nc.declare_dram_parameter("out", [TOK, ISIZE], F32,
                                        isOutput=True)
    with FixedTileContext(nc) as tc:
        with ExitStack() as ctx:
            bld = Builder(nc, tc, ctx)
            acc = _emit_graph(bld, np_in, routes, core_mask_bias)
            _emit_final(bld, acc, out_hdl, np.asarray(np_in['out_g']),
                        np.asarray(np_in['out_beta']))
            uploads = bld.uploads
    _hoist_excess_waits(nc)
    in_maps = [{nm: arrs[i] for nm, arrs in uploads.items()}
               for i in range(NCORE)]
    res = run_bass_kernel_spmd(nc, in_maps, core_ids=list(range(NCORE)),
                               trace=trace, **run_kwargs)
    out = np.concatenate([res.results[i]['out'] for i in range(NCORE)], 0)
    return out.reshape(B, SLEN, ISIZE).astype(np.float32), res


def kernel(**inputs):
    out, _ = _build_and_run(inputs)
    return out



# revision 3
# speedup vs baseline: 1.0003x; 1.0003x over previous
"""Trainium2 Bass kernel for nn_DecoderLayer_60060822667509.

Data-parallel over the 4096 tokens (512/core on 8 cores). Routing
(host-side argmax on small logits, mirroring the reference's .item()
syncs) is computed from the actual inputs at call time and a
specialized Bass/Tile program is emitted for the selected DAG.

Activations live feature-major on-chip ([128 features, NFC chunks, TOK
tokens]) so matmul outputs feed the next matmul's moving operand with
no transposes. LayerNorm affines, selection softmax weights and node
activation weights are folded into weight matrices host-side; residual
scalars ride along symbolically on each value. Attention (act 0) keys/
values are exchanged between the two cores sharing a batch via an
AllGather pair group.
"""
import numpy as np
import ml_dtypes
from contextlib import ExitStack

import concourse.bass as bass
import concourse.tile as tile
from concourse import mybir
from concourse.bass import ts
from concourse.bass_utils import run_bass_kernel_spmd
from concourse.masks import make_identity

F32 = mybir.dt.float32
BF16 = mybir.dt.bfloat16
AF = mybir.ActivationFunctionType
ALU = mybir.AluOpType

ISIZE = 512
NHEAD = 8
DH = ISIZE // NHEAD  # 64
NNOD = 8
MAXP = 5
TAU = 1.0
EPS = 1e-6
B = 4
SLEN = 1024
NCORE = 8
TOK = (B * SLEN) // NCORE  # 512 tokens per core
NFC = ISIZE // 128  # feature chunks
NTT = TOK // 128    # token tiles


# ---------------------------------------------------------------------------
# Host-side routing (mirrors reference._routing exactly)
# ---------------------------------------------------------------------------

def _qmask(nsrc):
    m = np.zeros((nsrc, 5), bool)
    m[0, :] = True
    return m.reshape(-1)


def _routing(node_p, edge_p):
    node_p = np.asarray(node_p)
    edge_p = np.asarray(edge_p)
    routes, lind = [], 0
    for c in range(NNOD):
        nsrc = min(c + 2, MAXP)
        snode = c - nsrc
        ep = edge_p[:, lind:lind + nsrc, :].reshape(3, -1)
        qm = _qmask(nsrc)
        nact = int(np.argmax(node_p[c]))
        qsel = int(np.argmax(np.where(qm, -np.inf, ep[0])))
        r = dict(lind=lind, nsrc=nsrc, snode=snode, act=nact, q=qsel, k=None,
                 v=None, ktype=None, km=None, vmode=None)
        if nact < 7:
            km = qm if nact > 0 else None
            kl = ep[1] if km is None else np.where(km, -np.inf, ep[1])
            r['k'] = int(np.argmax(kl))
            r['km'] = km
            r['ktype'] = -2 if r['k'] // 5 == 0 else -1
            if nact < 5:
                if nact == 0 and r['ktype'] == -2:
                    r['v'] = int(np.argmax(ep[2][:5]))
                    r['vmode'] = 'first5'
                else:
                    vl = ep[2] if km is None else np.where(km, -np.inf, ep[2])
                    r['v'] = int(np.argmax(vl))
                    r['vmode'] = 'full'
        routes.append(r)
        lind += nsrc
    return routes


def _softmax_np(x):
    x = np.asarray(x, np.float64)
    e = np.exp(x - x.max())
    return e / e.sum()


def _selw_np(logits, mask, sel):
    logits = np.asarray(logits, np.float64)
    if mask is not None:
        logits = np.where(np.asarray(mask), -np.inf, logits)
    return float(_softmax_np(logits / TAU)[sel])


# ---------------------------------------------------------------------------
# TileContext with a walrus-compatible tail drain: this compiler build
# rejects sem waits on SP Drain/NoOp (TPB_CTRL has no wait slots), so
# emit the end-of-kernel waits as standalone wait_ge instructions.
# ---------------------------------------------------------------------------

class FixedTileContext(tile.TileContext):
    def _drain_and_barrier(self, tick_clock, wait_clock):
        nc = self.nc
        clock = list(tick_clock.global_clock)
        for p, sem in sorted(self.sems.allocated().items()):
            c = clock[p]
            if c > 0:
                mult = 16 if sem.name.startswith("DMA") else 1
                nc.sync.wait_ge(sem, c * mult)
        nc.sync.drain()
        nc.all_engine_barrier()
        popped = nc._tile_sem_poison_stack.pop()
        assert popped is self._sem_poison
        nc.clear_and_free_semaphores(list(self.sems.allocated().values()))
        nc.all_engine_barrier()


# ---------------------------------------------------------------------------
# Device-tensor / value abstractions
# ---------------------------------------------------------------------------

class DT:
    """A per-core feature-major tensor: [128 part, NFC, TOK].
    Tiles can be spilled to DRAM and reloaded on demand (DTs are
    write-once, so a spill copy stays valid forever)."""
    def __init__(self, bld):
        self.bld = bld
        self.f32 = None
        self.bf = None
        self.spill = {}

    def _load(self, attr):
        b = self.bld
        dt_ = F32 if attr == "f32" else BF16
        t = b.acq([128, NFC, TOK], dt_)
        b.nc.sync.dma_start(t[:, :, :], self.spill[attr][:, :, :])
        setattr(self, attr, t)
        return t

    def need_bf(self):
        if self.bf is None:
            if "bf" in self.spill:
                return self._load("bf")
            if self.f32 is None and "f32" in self.spill:
                self._load("f32")
            assert self.f32 is not None
            b = self.bld
            self.bf = b.acq([128, NFC, TOK], BF16)
            for fc in range(NFC):
                b.nc.vector.tensor_copy(self.bf[:, fc, :], self.f32[:, fc, :])
        return self.bf

    def need_f32(self):
        if self.f32 is None:
            if "f32" in self.spill:
                return self._load("f32")
            if self.bf is None and "bf" in self.spill:
                self._load("bf")
            assert self.bf is not None
            b = self.bld
            self.f32 = b.acq([128, NFC, TOK], F32)
            for fc in range(NFC):
                b.nc.vector.tensor_copy(self.f32[:, fc, :], self.bf[:, fc, :])
        return self.f32

    def do_spill(self):
        b = self.bld
        for attr in ("f32", "bf"):
            t = getattr(self, attr)
            if t is None:
                continue
            if attr not in self.spill:
                d = b.nc.dram_tensor(
                    b.tag("sp"), [128, NFC, TOK],
                    F32 if attr == "f32" else BF16)
                b.nc.sync.dma_start(d[:, :, :], t[:, :, :])
                self.spill[attr] = d
            b.rel_tile(t)
            setattr(self, attr, None)

    def any(self):
        """Whichever representation exists (no conversion pass); engines
        convert dtypes on read."""
        if self.f32 is not None:
            return self.f32
        if self.bf is not None:
            return self.bf
        if "bf" in self.spill:
            return self._load("bf")
        return self._load("f32")

    def tiles(self):
        return [t for t in (self.f32, self.bf) if t is not None]


class Val:
    """dt scaled by host scalar `mult`; unit=True => per-token zero mean,
    unit variance (LayerNorm output)."""
    def __init__(self, dt, mult=1.0, unit=False):
        self.dt = dt
        self.mult = float(mult)
        self.unit = unit


class Builder:
    def __init__(self, nc, tc, ctx):
        self.nc = nc
        self.tc = tc
        self.uploads = {}
        self.n_tag = 0
        self.act_pool = ctx.enter_context(tc.tile_pool(name="act", bufs=1))
        self.w_pool = ctx.enter_context(tc.tile_pool(name="w", bufs=2))
        self.small_pool = ctx.enter_context(tc.tile_pool(name="small", bufs=1))
        self.ps_pool = ctx.enter_context(
            tc.tile_pool(name="ps", bufs=6, space="PSUM"))
        self.ps_stat = ctx.enter_context(
            tc.tile_pool(name="pstat", bufs=2, space="PSUM"))
        self.ln_cache = {}
        self.live_provider = lambda: set()
        # tile lifetime management
        self.freelist = {}
        self.meta = {}
        self.released = set()
        self.window = []
        # constants
        self.ident_f32 = self.small_pool.tile([128, 128], F32, tag="idf")
        make_identity(nc, self.ident_f32)
        self.ident_bf = self.small_pool.tile([128, 128], BF16, tag="idb")
        make_identity(nc, self.ident_bf)
        self.ones_bf = self.small_pool.tile([128, 1], BF16, tag="ones")
        nc.vector.memset(self.ones_bf, 1.0)
        self.ones_row_f32 = self.small_pool.tile([1, 128], F32, tag="onesr")
        nc.vector.memset(self.ones_row_f32, 1.0)
        self.ones_row_bf = self.small_pool.tile([1, 128], BF16, tag="onesrb")
        nc.vector.memset(self.ones_row_bf, 1.0)
        self.stats_cache = {}

    def tag(self, kind="t"):
        self.n_tag += 1
        return f"{kind}{self.n_tag}"

    # -- recyclable SBUF tiles ----------------------------------------------
    def acq(self, shape, dtype, kind="a"):
        key = (tuple(shape), str(dtype))
        lst = self.freelist.get(key)
        tag = lst.pop() if lst else self.tag(kind)
        t = self.act_pool.tile(list(shape), dtype, tag=tag)
        self.meta[id(t)] = (key, tag)
        self.window.append(t)
        return t

    def rel_tile(self, t):
        if t is None:
            return
        i = id(t)
        if i in self.released or i not in self.meta:
            return
        key, tag = self.meta[i]
        self.freelist.setdefault(key, []).append(tag)
        self.released.add(i)

    def flush(self, keep_vals=(), keep_tiles=()):
        keep = set(self.live_provider())
        for v in keep_vals:
            if v is not None:
                for t in v.dt.tiles():
                    keep.add(id(t))
        for t in keep_tiles:
            if t is not None:
                keep.add(id(t))
        for t in self.window:
            if id(t) not in keep:
                self.rel_tile(t)
        self.window = [t for t in self.window if id(t) in keep]

    def const_col(self, value, parts=128):
        key = (float(value), parts)
        if not hasattr(self, "_cc_cache"):
            self._cc_cache = {}
        if key not in self._cc_cache:
            t = self.small_pool.tile([parts, 1], F32, tag=self.tag("cc"))
            self.nc.vector.memset(t, float(value))
            self._cc_cache[key] = t
        return self._cc_cache[key]

    # -- host->device uploads -----------------------------------------------
    def upload(self, base, arrs, shape, dtype):
        name = f"{base}{len(self.uploads)}"
        if not isinstance(arrs, list):
            arrs = [arrs] * NCORE
        self.uploads[name] = [np.ascontiguousarray(a) for a in arrs]
        return self.nc.declare_dram_parameter(name, list(shape), dtype,
                                              isOutput=False)

    def upload_weight(self, w_np):
        """w_np [512, 512] -> bf16 SBUF tile [128, NFC, 512]."""
        arr = np.ascontiguousarray(
            np.asarray(w_np, np.float32).reshape(NFC, 128, ISIZE)
            .transpose(1, 0, 2)).astype(ml_dtypes.bfloat16)
        hdl = self.upload("w", arr, [128, NFC, ISIZE], BF16)
        t = self.w_pool.tile([128, NFC, ISIZE], BF16, tag="w")
        self.nc.sync.dma_start(t[:, :, :], hdl[:, :, :])
        return t

    def upload_bias(self, b_np):
        """b_np [512] -> SBUF [128, NFC] f32 (per-partition scalars)."""
        arr = np.ascontiguousarray(
            np.asarray(b_np, np.float32).reshape(NFC, 128).transpose(1, 0))
        hdl = self.upload("b", arr, [128, NFC], F32)
        t = self.small_pool.tile([128, NFC], F32, tag=self.tag("bias"))
        self.nc.sync.dma_start(t[:, :], hdl[:, :])
        return t

    # -- emission helpers ----------------------------------------------------
    def load_input_fm(self, hdl):
        """DRAM [TOK, 512] bf16 token-major -> feature-major DT (bf16)."""
        nc = self.nc
        dt = DT(self)
        dt.bf = self.acq([128, NFC, TOK], BF16)
        tok_tiles = []
        for tt in range(NTT):
            t = self.acq([128, ISIZE], BF16)
            nc.sync.dma_start(t[:, :], hdl[ts(tt, 128), :])
            tok_tiles.append(t)
        for fc in range(NFC):
            ps = self.ps_pool.tile([128, TOK], BF16, tag="ps")
            for tt in range(NTT):
                nc.tensor.transpose(ps[:, ts(tt, 128)],
                                    tok_tiles[tt][:, ts(fc, 128)],
                                    self.ident_bf)
            nc.scalar.activation(dt.bf[:, fc, :], ps[:, :], AF.Identity)
        return Val(dt, 1.0, False)

    def mm_psums(self, parts):
        """Matmuls accumulating into NFC psum tiles [128, TOK]; returns them.
        parts: list of (Val, W_np[512,512]); Val.mult folded into W."""
        nc = self.nc
        wts = [self.upload_weight(np.asarray(w, np.float64) * v.mult)
               for v, w in parts]
        rhs = [v.dt.need_bf() for v, _ in parts]
        psums = []
        for mc in range(NFC):
            ps = self.ps_pool.tile([128, TOK], F32, tag="ps")
            first = True
            for wi, (wt, r) in enumerate(zip(wts, rhs)):
                for kc in range(NFC):
                    nc.tensor.matmul(ps[:, :], wt[:, kc, ts(mc, 128)],
                                     r[:, kc, :], start=first,
                                     stop=(wi == len(wts) - 1 and
                                           kc == NFC - 1))
                    first = False
            psums.append(ps)
        return psums

    def matmul_fm(self, parts, bias_np=None, epi="identity", epi_scale=1.0,
                  out_f32=True, out_bf=False):
        """epi( sum_i (mult_i*x_i) @ W_i + bias ) -> Val(mult=1).
        epi in {identity, relu, gelu}; epi_scale pre-scales inside relu."""
        nc = self.nc
        psums = self.mm_psums(parts)
        bias_t = None
        if bias_np is not None and np.any(bias_np):
            bias_t = self.upload_bias(
                np.asarray(bias_np, np.float64) *
                (epi_scale if epi == "relu" else 1.0))
        dt = DT(self)
        if out_f32:
            dt.f32 = self.acq([128, NFC, TOK], F32)
        if out_bf:
            dt.bf = self.acq([128, NFC, TOK], BF16)
        func = {"identity": AF.Identity, "relu": AF.Relu,
                "gelu": AF.Gelu_apprx_tanh}[epi]
        for mc, ps in enumerate(psums):
            bias_ap = bias_t[:, mc:mc + 1] if bias_t is not None else 0.0
            scale = epi_scale if epi == "relu" else 1.0
            tgt = dt.f32 if dt.f32 is not None else dt.bf
            nc.scalar.activation(tgt[:, mc, :], ps[:, :], func,
                                 bias=bias_ap, scale=scale)
            if dt.f32 is not None and dt.bf is not None:
                nc.vector.tensor_copy(dt.bf[:, mc, :], dt.f32[:, mc, :])
        return Val(dt, 1.0, False)

    def act_pass(self, val, func, scale=1.0):
        """Elementwise ACT func(scale*mult*x) -> Val(mult=1), bf16."""
        nc = self.nc
        src = val.dt.any()
        dt = DT(self)
        dt.bf = self.acq([128, NFC, TOK], BF16)
        for fc in range(NFC):
            nc.scalar.activation(dt.bf[:, fc, :], src[:, fc, :], func,
                                 scale=float(scale * val.mult))
        return Val(dt, 1.0, False)

    def axpy(self, a, b, out_bf=False):
        """a.mult*a + b.mult*b (one DVE pass)."""
        nc = self.nc
        if abs(a.mult) > abs(b.mult):
            a, b = b, a
        dt = DT(self)
        t = self.acq([128, NFC, TOK], BF16 if out_bf else F32)
        if out_bf:
            dt.bf = t
        else:
            dt.f32 = t
        aa, bb = a.dt.any(), b.dt.any()
        for fc in range(NFC):
            nc.vector.scalar_tensor_tensor(
                t[:, fc, :], aa[:, fc, :], float(a.mult / b.mult),
                bb[:, fc, :], op0=ALU.mult, op1=ALU.add)
        return Val(dt, b.mult, False)

    def mul_vals(self, a, b, extra=1.0):
        nc = self.nc
        dt = DT(self)
        dt.f32 = self.acq([128, NFC, TOK], F32)
        aa, bb = a.dt.any(), b.dt.any()
        for fc in range(NFC):
            nc.vector.tensor_mul(dt.f32[:, fc, :], aa[:, fc, :],
                                 bb[:, fc, :])
        return Val(dt, a.mult * b.mult * extra, False)

    def add_psum_resid(self, resid, resid_scale, psums):
        """resid.t * resid_scale + psum (per-chunk fused passes)."""
        nc = self.nc
        dt = DT(self)
        dt.f32 = self.acq([128, NFC, TOK], F32)
        rt = resid.dt.any()
        for mc, ps in enumerate(psums):
            nc.vector.scalar_tensor_tensor(
                dt.f32[:, mc, :], rt[:, mc, :], float(resid_scale),
                ps[:, :], op0=ALU.mult, op1=ALU.add)
        return Val(dt, 1.0, False)

    def ln_stats(self, val):
        """Per-token LN statistics of a feature-major value, for fused-LN
        matmuls: returns (m_bf [1,TOK] bf16, rb_sb [128,TOK] bf16 broadcast
        of rstd). Cached per underlying tensor."""
        key = (id(val.dt), round(float(val.mult), 12))
        c = self.stats_cache.get(key)
        if c is not None:
            return c[1], c[2]
        nc = self.nc
        xbf = val.dt.need_bf()
        x2 = self.acq([128, NFC, TOK], BF16)
        for fc in range(NFC):
            nc.vector.tensor_mul(x2[:, fc, :], xbf[:, fc, :], xbf[:, fc, :])
        m_ps = self.ps_stat.tile([1, TOK], F32, tag="st")
        s2_ps = self.ps_stat.tile([1, TOK], F32, tag="st")
        for kc in range(NFC):
            nc.tensor.matmul(m_ps[:, :], self.ones_bf[:, :], xbf[:, kc, :],
                             start=(kc == 0), stop=(kc == NFC - 1))
        for kc in range(NFC):
            nc.tensor.matmul(s2_ps[:, :], self.ones_bf[:, :], x2[:, kc, :],
                             start=(kc == 0), stop=(kc == NFC - 1))
        sm = self.acq([1, 3 * TOK], F32)
        s0, s1, s2 = (sm[:, ts(i, TOK)] for i in range(3))
        nc.vector.tensor_scalar_mul(s0, m_ps[:, :], 1.0 / ISIZE)   # mean
        nc.vector.scalar_tensor_tensor(s2, s0, -1.0, s0,
                                       op0=ALU.mult, op1=ALU.mult)
        nc.vector.scalar_tensor_tensor(s1, s2_ps[:, :], 1.0 / ISIZE, s2,
                                       op0=ALU.mult, op1=ALU.add)   # var
        epsp = EPS / (val.mult * val.mult)
        nc.scalar.activation(s2, s1, AF.Ln, bias=self.const_col(epsp, 1))
        nc.scalar.activation(s1, s2, AF.Exp, scale=-0.5)            # rstd
        m_bf = self.acq([1, TOK], BF16)
        r_bf = self.acq([1, TOK], BF16)
        nc.vector.tensor_copy(m_bf[:, :], s0)
        nc.vector.tensor_copy(r_bf[:, :], s1)
        rb_ps = self.ps_stat.tile([128, TOK], F32, tag="st")
        nc.tensor.matmul(rb_ps[:, :], self.ones_row_bf[:, :], r_bf[:, :],
                         start=True, stop=True)
        rb_sb = self.acq([128, TOK], BF16)
        nc.scalar.activation(rb_sb[:, :], rb_ps[:, :], AF.Identity)
        self.rel_tile(x2)
        self.rel_tile(sm)
        self.rel_tile(r_bf)
        self.stats_cache[key] = (val.mult, m_bf, rb_sb)
        return m_bf, rb_sb

    def matmul_fm_ln(self, val, w_eff, bias_np=None, out_f32=False,
                     out_bf=True):
        """LNraw(val) @ w_eff + bias, with the matmuls running on the RAW
        activations: mean is subtracted inside PSUM via a K=1 matmul with
        the column sums of w_eff, and rstd is applied in the PSUM->SBUF
        epilogue (both commute with the contraction)."""
        nc = self.nc
        m_bf, rb_sb = self.ln_stats(val)
        wbf = np.asarray(w_eff, np.float32).astype(ml_dtypes.bfloat16)
        wt = self.upload_weight(wbf)
        wcs = np.ascontiguousarray(
            -wbf.astype(np.float32).sum(axis=0)[None, :]
        ).astype(ml_dtypes.bfloat16)
        hw = self.upload("wc", wcs, [1, ISIZE], BF16)
        wcs_t = self.acq([1, ISIZE], BF16)
        nc.gpsimd.dma_start(wcs_t[:, :], hw[:, :])
        xbf = val.dt.need_bf()
        dt = DT(self)
        if out_bf:
            dt.bf = self.acq([128, NFC, TOK], BF16)
        if out_f32:
            dt.f32 = self.acq([128, NFC, TOK], F32)
        bias_t = self.upload_bias(bias_np) \
            if bias_np is not None and np.any(bias_np) else None
        for mc in range(NFC):
            ps = self.ps_pool.tile([128, TOK], F32, tag="ps")
            for kc in range(NFC):
                nc.tensor.matmul(ps[:, :], wt[:, kc, ts(mc, 128)],
                                 xbf[:, kc, :], start=(kc == 0), stop=False)
            nc.tensor.matmul(ps[:, :], wcs_t[0:1, ts(mc, 128)], m_bf[:, :],
                             start=False, stop=True)
            tgt = dt.bf if dt.bf is not None else dt.f32
            nc.vector.scalar_tensor_tensor(
                tgt[:, mc, :], ps[:, :], 1.0, rb_sb[:, :],
                op0=ALU.mult, op1=ALU.mult)
            if dt.bf is not None and dt.f32 is not None:
                nc.vector.tensor_copy(dt.f32[:, mc, :], dt.bf[:, mc, :])
            if bias_t is not None:
                for t in dt.tiles():
                    nc.scalar.activation(t[:, mc, :], t[:, mc, :],
                                         AF.Identity,
                                         bias=bias_t[:, mc:mc + 1])
        self.rel_tile(wcs_t)
        return Val(dt, 1.0, False)

    def ln_fm(self, val, out_f32=False, out_bf=True):
        """Feature-major LNraw; scale-invariant up to eps (folded exactly
        into eps'). Unit-LN input collapses to a host scalar."""
        if val.unit:
            kappa = 1.0 / np.sqrt(1.0 + EPS / (val.mult * val.mult))
            return Val(val.dt, kappa, True)
        key = id(val.dt)
        if key in self.ln_cache:
            return self.ln_cache[key][1]
        nc = self.nc
        xs = val.dt.any()
        xbf = val.dt.need_bf()
        x2 = self.acq([128, NFC, TOK], BF16)
        nc.vector.tensor_mul(x2[:, :, :], xs[:, :, :], xs[:, :, :])
        m_ps = self.ps_stat.tile([1, TOK], F32, tag="st")
        s2_ps = self.ps_stat.tile([1, TOK], F32, tag="st")
        for kc in range(NFC):
            nc.tensor.matmul(m_ps[:, :], self.ones_bf[:, :], xbf[:, kc, :],
                             start=(kc == 0), stop=(kc == NFC - 1))
        for kc in range(NFC):
            nc.tensor.matmul(s2_ps[:, :], self.ones_bf[:, :], x2[:, kc, :],
                             start=(kc == 0), stop=(kc == NFC - 1))
        sm = self.acq([1, 3 * TOK], F32)
        s0, s1, s2 = (sm[:, ts(i, TOK)] for i in range(3))
        nc.vector.tensor_scalar_mul(s0, m_ps[:, :], 1.0 / ISIZE)   # mean
        nc.vector.tensor_scalar_mul(s1, s2_ps[:, :], 1.0 / ISIZE)  # E[x^2]
        nc.vector.scalar_tensor_tensor(s2, s0, -1.0, s0,
                                       op0=ALU.mult, op1=ALU.mult)  # -mean^2
        nc.vector.tensor_add(s1, s1, s2)                            # var
        epsp = EPS / (val.mult * val.mult)
        nc.scalar.activation(s2, s1, AF.Ln, bias=self.const_col(epsp, 1))
        nc.scalar.activation(s1, s2, AF.Exp, scale=-0.5)            # rstd
        nc.vector.tensor_mul(s2, s0, s1)                            # mean*rstd
        rstd, mr = s1, s2
        rb_ps = self.ps_stat.tile([128, TOK], F32, tag="st")
        mrb_ps = self.ps_stat.tile([128, TOK], F32, tag="st")
        nc.tensor.matmul(rb_ps[:, :], self.ones_row_f32[:, :], rstd,
                         start=True, stop=True)
        nc.tensor.matmul(mrb_ps[:, :], self.ones_row_f32[:, :], mr,
                         start=True, stop=True)
        rb = self.acq([128, TOK], BF16)
        mrb = self.acq([128, TOK], BF16)
        nc.scalar.activation(rb[:, :], rb_ps[:, :], AF.Identity)
        nc.scalar.activation(mrb[:, :], mrb_ps[:, :], AF.Identity)
        dt = DT(self)
        u = self.acq([128, NFC, TOK], BF16)
        for fc in range(NFC):
            nc.vector.tensor_mul(u[:, fc, :], xs[:, fc, :], rb[:, :])
        targets = []
        if out_bf:
            dt.bf = self.acq([128, NFC, TOK], BF16)
            targets.append(dt.bf)
        if out_f32:
            dt.f32 = self.acq([128, NFC, TOK], F32)
            targets.append(dt.f32)
        for t in targets:
            for fc in range(NFC):
                nc.vector.scalar_tensor_tensor(
                    t[:, fc, :], u[:, fc, :], 1.0, mrb[:, :],
                    op0=ALU.mult, op1=ALU.subtract)
        out = Val(dt, 1.0, True)
        self.ln_cache[key] = (val.dt, out)
        return out

    # -- multi-head attention (act 0) ---------------------------------------
    def emit_mha(self, qv, kv, vv, nW, nb, ng, nbe, aw, core_mask_arrs):
        nc = self.nc
        mid = self.tag("mha")
        w0 = np.asarray(ng, np.float64)[:, None] * np.asarray(nW[0], np.float64)
        b0 = np.asarray(nbe, np.float64) @ np.asarray(nW[0], np.float64) \
            + np.asarray(nb[0], np.float64)
        if qv.unit:
            qn = self.ln_fm(qv)
            qh = self.matmul_fm([(qn, w0)], bias_np=b0, out_f32=False,
                                out_bf=True)
        else:
            qh = self.matmul_fm_ln(qv, w0, bias_np=b0, out_f32=False,
                                   out_bf=True)
        kh = self.matmul_fm([(kv, np.asarray(nW[1], np.float64))],
                            bias_np=np.asarray(nb[1], np.float64),
                            out_f32=False, out_bf=True)
        # vh token-major [128 tok, (h, dh)] with a trailing ones column
        w2t = self.upload_weight(np.asarray(nW[2], np.float64) * vv.mult)
        vbf = vv.dt.need_bf()
        b2 = np.asarray(nb[2], np.float64)
        b2_row = None
        if np.any(b2):
            hb = self.upload("vb", b2.astype(np.float32)[None, :],
                             [1, ISIZE], F32)
            b2_row = self.small_pool.tile([1, ISIZE], F32, tag=self.tag("vb"))
            nc.sync.dma_start(b2_row[:, :], hb[:, :])
        vht = self.acq([128, NTT, NHEAD, DH + 1], BF16)
        for tt in range(NTT):
            ps = self.ps_pool.tile([128, ISIZE], F32, tag="ps")
            for kc in range(NFC):
                nc.tensor.matmul(ps[:, :], vbf[:, kc, ts(tt, 128)],
                                 w2t[:, kc, :], start=(kc == 0),
                                 stop=(kc == NFC - 1 and b2_row is None))
            if b2_row is not None:
                nc.tensor.matmul(ps[:, :], self.ones_row_f32[:, :],
                                 b2_row[:, :], start=False, stop=True)
            nc.scalar.activation(
                vht[:, tt, :, 0:DH],
                ps[:, :].rearrange("p (h d) -> p h d", h=NHEAD),
                AF.Identity)
        nc.vector.memset(vht[:, :, :, DH], 1.0)
        # pairwise AllGather of kh (feature-major) and vht (token-major)
        kh_loc = nc.dram_tensor(f"khl{mid}", [128, NFC, TOK], BF16)
        vh_loc = nc.dram_tensor(f"vhl{mid}", [128, NTT, NHEAD, DH + 1], BF16)
        kh_g = nc.dram_tensor(f"khg{mid}", [2, 128, NFC, TOK], BF16)
        vh_g = nc.dram_tensor(f"vhg{mid}", [2, 128, NTT, NHEAD, DH + 1],
                              BF16)
        nc.sync.dma_start(kh_loc[:, :, :], kh.dt.bf[:, :, :])
        nc.sync.dma_start(vh_loc[:, :, :, :], vht[:, :, :, :])
        groups = [[0, 1], [2, 3], [4, 5], [6, 7]]
        nc.gpsimd.collective_compute(
            "AllGather", ALU.bypass, replica_groups=groups,
            ins=[kh_loc[:, :, :]], outs=[kh_g[:, :, :, :]])
        nc.gpsimd.collective_compute(
            "AllGather", ALU.bypass, replica_groups=groups,
            ins=[vh_loc[:, :, :, :]], outs=[vh_g[:, :, :, :, :]])
        khg = self.acq([128, 2, NFC, TOK], BF16)
        vhg = self.acq([128, 2, NTT, NHEAD, DH + 1], BF16)
        for r in range(2):
            nc.sync.dma_start(khg[:, r, :, :], kh_g[r, :, :, :])
            nc.sync.dma_start(vhg[:, r, :, :, :], vh_g[r, :, :, :, :])
        self.flush(keep_vals=[qv, kv, vv, qh], keep_tiles=[khg, vhg])
        maskb = None
        if core_mask_arrs is not None:
            hb = self.upload("mb", core_mask_arrs, [128, 2 * NTT], F32)
            maskb = self.small_pool.tile([128, 2 * NTT], F32,
                                         tag=self.tag("mb"))
            nc.sync.dma_start(maskb[:, :], hb[:, :])
        qhbf = qh.dt.bf
        oTn = DT(self)
        oTn.bf = self.acq([128, NFC, TOK], BF16)
        scale = 1.0 / float(np.sqrt(DH))
        for h in range(NHEAD):
            po = DH * (h % 2)
            fc = h // 2
            att = self.ps_stat.tile([DH + 1, TOK], F32, tag="st")
            for kc8 in range(2 * NTT):
                r, tl = kc8 // NTT, kc8 % NTT
                sT = self.ps_pool.tile([128, TOK], F32, tag="ps")
                nc.tensor.matmul(sT[:, :],
                                 khg[po:po + DH, r, fc, ts(tl, 128)],
                                 qhbf[po:po + DH, fc, :],
                                 start=True, stop=True)
                bias_ap = maskb[:, kc8:kc8 + 1] if maskb is not None else 0.0
                exp_sb = self.acq([128, TOK], BF16)
                nc.scalar.activation(exp_sb[:, :], sT[:, :], AF.Exp,
                                     bias=bias_ap, scale=scale)
                nc.tensor.matmul(att[:, :],
                                 vhg[:, r, tl, h, :],
                                 exp_sb[:, :], start=(kc8 == 0),
                                 stop=(kc8 == 2 * NTT - 1))
                self.rel_tile(exp_sb)
            # normalize: recip(rowsum) broadcast over the head's partitions
            rs_sb = self.acq([1, TOK], F32)
            nc.scalar.activation(rs_sb[:, :], att[DH:DH + 1, :], AF.Ln)
            nc.scalar.activation(rs_sb[:, :], rs_sb[:, :], AF.Exp, scale=-1.0)
            rb_ps = self.ps_stat.tile([DH, TOK], F32, tag="st")
            nc.tensor.matmul(rb_ps[:, :], self.ones_row_f32[:, 0:DH],
                             rs_sb[:, :], start=True, stop=True)
            rb_sb = self.acq([128, TOK], F32)
            nc.scalar.activation(rb_sb[0:DH, :], rb_ps[:, :], AF.Identity)
            nc.vector.tensor_mul(oTn.bf[po:po + DH, fc, :], att[0:DH, :],
                                 rb_sb[0:DH, :])
            self.rel_tile(rs_sb)
            self.rel_tile(rb_sb)
        self.flush(keep_vals=[qv], keep_tiles=list(oTn.tiles()))
        b3 = np.asarray(nb[3], np.float64)
        w3 = aw * np.asarray(nW[3], np.float64)
        if np.any(b3):
            pr = self.matmul_fm([(Val(oTn, 1.0), w3)], bias_np=aw * b3,
                                out_f32=True)
            return self.axpy(Val(qv.dt, qv.mult * aw, False),
                             Val(pr.dt, 1.0, False))
        psums = self.mm_psums([(Val(oTn, 1.0), w3)])
        return self.add_psum_resid(qv, aw * qv.mult, psums)




# ---------------------------------------------------------------------------
# Walrus-compat post-pass: this compiler build supports at most one sync
# wait on most engine instructions (none on SP control ops). Hoist excess
# waits onto standalone InstEventSemaphore instructions inserted before.
# ---------------------------------------------------------------------------

_NO_HOIST = ("InstEventSemaphore", "InstAllEngineBarrier",
             "InstCollectiveCompute")


def _hoist_excess_waits(nc):
    n = 0
    for f in nc.m.functions:
        for bb in f.blocks:
            out = []
            changed = False
            for inst in bb.instructions:
                tname = type(inst).__name__
                si = inst.sync_info
                if si is not None and tname not in _NO_HOIST:
                    waits = list(si.on_wait)
                    limit = 0 if tname in ("InstDrain", "InstNoOp") else 1
                    if len(waits) > limit:
                        for w in waits[:len(waits) - limit]:
                            n += 1
                            ni = mybir.InstEventSemaphore(
                                name=f"I-hoist{n}", ins=[], outs=[])
                            ni.engine = inst.engine
                            ni.sync_info = mybir.SyncInfo(on_wait=[w],
                                                          on_update=[])
                            out.append(ni)
                        si.on_wait = waits[len(waits) - limit:]
                        changed = True
                out.append(inst)
            if changed:
                bb.instructions = out
    return n


# ===========================================================================
# NEW FAST PATH (v2): all-bf16 feature-major, materialized LN, single
# activation table (ln/exp only; gelu/sigmoid via exp + DVE reciprocal),
# grouped-psum GEMMs with host-side linear-term merging, no spills.
# Falls back to the old emitter on any unsupported structure (e.g. MHA).
# ===========================================================================

class NVal:
    """bf16 tile [128, NFC, TOK]; actual value = mult * tile.
    lin=(src_tile, A, scale, b): actual = src @ (A if A is not None else
    scale*I) + b, enabling host-side weight folding in linear consumers."""
    __slots__ = ("t", "mult", "unit", "lin")

    def __init__(self, t, mult=1.0, unit=False, lin=None):
        self.t = t
        self.mult = float(mult)
        self.unit = unit
        self.lin = lin


def _kappa(mult):
    return 1.0 / np.sqrt(1.0 + EPS / (mult * mult))


class B2:
    def __init__(self, nc, tc, ctx):
        self.nc = nc
        self.tc = tc
        self.uploads = {}
        self.n_tag = 0
        self.order = {}
        self.ocnt = 0
        self.window = []
        self.act_pool = ctx.enter_context(tc.tile_pool(name="a2", bufs=1))
        self.w_pool = ctx.enter_context(tc.tile_pool(name="w2", bufs=1))
        self.small = ctx.enter_context(tc.tile_pool(name="s2", bufs=1))
        self.ps_pool = ctx.enter_context(
            tc.tile_pool(name="p2", bufs=6, space="PSUM"))
        self.ps_stat = ctx.enter_context(
            tc.tile_pool(name="q2", bufs=2, space="PSUM"))
        self.freelist = {}
        self.meta = {}
        self.released = set()
        self.ln_cache = {}
        self.pending_ln = []
        self.ident_bf = self.small.tile([128, 128], BF16, tag="id2b")
        make_identity(nc, self.ident_bf)
        self.ones_col_bf = self.small.tile([128, 1], BF16, tag="o2c")
        nc.vector.memset(self.ones_col_bf, 1.0)
        self.ones_row_bf = self.small.tile([1, 128], BF16, tag="o2r")
        nc.vector.memset(self.ones_row_bf, 1.0)
        self._cc = {}

    def tag(self, kind="t"):
        self.n_tag += 1
        return f"{kind}{self.n_tag}"

    def acq(self, shape, dtype, kind="a"):
        key = (tuple(shape), str(dtype))
        lst = self.freelist.get(key)
        tag = lst.pop() if lst else self.tag(kind)
        t = self.act_pool.tile(list(shape), dtype, tag=tag)
        self.meta[id(t)] = (key, tag)
        self.ocnt += 1
        self.order[id(t)] = self.ocnt
        self.window.append(t)
        return t

    def rel_tile(self, t):
        if t is None:
            return
        i = id(t)
        if i in self.released or i not in self.meta:
            return
        key, tag = self.meta[i]
        self.freelist.setdefault(key, []).append(tag)
        self.released.add(i)

    def const_col(self, value, parts=128):
        key = (float(value), parts)
        if key not in self._cc:
            t = self.small.tile([parts, 1], F32, tag=self.tag("cc"))
            self.nc.vector.memset(t, float(value))
            self._cc[key] = t
        return self._cc[key]

    def upload(self, base, arrs, shape, dtype):
        name = f"{base}{len(self.uploads)}"
        if not isinstance(arrs, list):
            arrs = [arrs] * NCORE
        self.uploads[name] = [np.ascontiguousarray(a) for a in arrs]
        return self.nc.declare_dram_parameter(name, list(shape), dtype,
                                              isOutput=False)

    def upload_weight(self, w_np):
        arr = np.ascontiguousarray(
            np.asarray(w_np, np.float32).reshape(NFC, 128, ISIZE)
            .transpose(1, 0, 2)).astype(ml_dtypes.bfloat16)
        hdl = self.upload("w", arr, [128, NFC, ISIZE], BF16)
        t = self.w_pool.tile([128, NFC, ISIZE], BF16, tag=self.tag("w"))
        self.nc.sync.dma_start(t[:, :, :], hdl[:, :, :])
        return t

    def upload_bias(self, b_np):
        arr = np.ascontiguousarray(
            np.asarray(b_np, np.float32).reshape(NFC, 128).transpose(1, 0))
        hdl = self.upload("b", arr, [128, NFC], F32)
        t = self.small.tile([128, NFC], F32, tag=self.tag("bias"))
        self.nc.sync.dma_start(t[:, :], hdl[:, :])
        return t

    # -- input load: token-major DRAM -> feature-major bf16 -----------------
    def load_input2(self, hdl):
        nc = self.nc
        x = self.acq([128, NFC, TOK], BF16)
        toks = []
        for tt in range(NTT):
            t = self.acq([128, ISIZE], BF16)
            nc.sync.dma_start(t[:, :], hdl[ts(tt, 128), :])
            toks.append(t)
        psums = [self.ps_pool.tile([128, TOK], F32, tag=self.tag("ps"))
                 for _ in range(NFC)]
        for tt in range(NTT):
            for fc in range(NFC):
                nc.tensor.transpose(psums[fc][:, ts(tt, 128)],
                                    toks[tt][:, ts(fc, 128)], self.ident_bf)
        for fc in range(NFC):
            nc.scalar.activation(x[:, fc, :], psums[fc][:, :], AF.Identity)
        for t in toks:
            self.rel_tile(t)
        return NVal(x, 1.0, False)

    # -- GEMM ----------------------------------------------------------------
    def mm2(self, groups, bias_np=None, epi="identity", resid=None):
        """groups: list of groups; each group is a list of (tile, W_abs).
        result = epi(sum tile@W + bias) (or resid_val*resid_scale + psum).
        Group g's matmuls for ALL output chunks are emitted before group
        g+1's, so late-ready groups never block early ones."""
        nc = self.nc
        wts = [[(self.upload_weight(W), t) for (t, W) in g] for g in groups]
        bias_t = None
        if bias_np is not None and np.any(bias_np):
            assert resid is None
            bias_t = self.upload_bias(bias_np)
        psums = [self.ps_pool.tile([128, TOK], F32, tag=self.tag("ps"))
                 for _ in range(NFC)]
        started = [False] * NFC
        ng = len(wts)
        for gi, g in enumerate(wts):
            for mc in range(NFC):
                for wi, (wt, t) in enumerate(g):
                    for kc in range(NFC):
                        nc.tensor.matmul(
                            psums[mc][:, :], wt[:, kc, ts(mc, 128)],
                            t[:, kc, :], start=not started[mc],
                            stop=(gi == ng - 1 and wi == len(g) - 1 and
                                  kc == NFC - 1))
                        started[mc] = True
        out = self.acq([128, NFC, TOK], BF16)
        for mc in range(NFC):
            if resid is not None:
                rv, rs = resid
                nc.vector.scalar_tensor_tensor(
                    out[:, mc, :], rv.t[:, mc, :], float(rs * rv.mult),
                    psums[mc][:, :], op0=ALU.mult, op1=ALU.add)
            else:
                func = AF.Relu if epi == "relu" else AF.Identity
                bias_ap = bias_t[:, mc:mc + 1] if bias_t is not None else 0.0
                nc.scalar.activation(out[:, mc, :], psums[mc][:, :], func,
                                     bias=bias_ap)
        self.flush_pending()
        return NVal(out, 1.0, False)

    def flush_pending(self):
        pend, self.pending_ln = self.pending_ln, []
        for v in pend:
            self.ln2(v)

    # -- elementwise ---------------------------------------------------------
    def relu_of(self, val):
        """relu(val.actual) as (tile=relu(raw), mult) — valid for mult>0."""
        assert val.mult > 0
        nc = self.nc
        out = self.acq([128, NFC, TOK], BF16)
        for fc in range(NFC):
            nc.scalar.activation(out[:, fc, :], val.t[:, fc, :], AF.Relu)
        return NVal(out, val.mult, False)

    def sigmoid_of(self, val, extra_scale=1.0):
        """sigmoid(extra_scale * val.actual), via exp + DVE reciprocal
        (keeps the scalar engine on the ln/exp activation table)."""
        nc = self.nc
        e = self.acq([128, NFC, TOK], BF16)
        for fc in range(NFC):
            nc.scalar.activation(e[:, fc, :], val.t[:, fc, :], AF.Exp,
                                 scale=float(-extra_scale * val.mult))
        nc.vector.tensor_scalar_add(e[:, :, :], e[:, :, :], 1.0)
        r = self.acq([128, NFC, TOK], F32)
        nc.vector.reciprocal(r[:, :, :], e[:, :, :])
        self.rel_tile(e)
        return r  # raw f32 tile, sigmoid value itself

    def gelu_of(self, val):
        """gelu(val.actual) ~= val.actual * sigmoid(1.702*val.actual)."""
        nc = self.nc
        sig = self.sigmoid_of(val, 1.702)
        out = self.acq([128, NFC, TOK], BF16)
        nc.vector.tensor_mul(out[:, :, :], val.t[:, :, :], sig[:, :, :])
        self.rel_tile(sig)
        return NVal(out, val.mult, False)

    def axpy2(self, a, b):
        nc = self.nc
        if abs(a.mult) > abs(b.mult):
            a, b = b, a
        out = self.acq([128, NFC, TOK], BF16)
        nc.vector.scalar_tensor_tensor(
            out[:, :, :], a.t[:, :, :], float(a.mult / b.mult),
            b.t[:, :, :], op0=ALU.mult, op1=ALU.add)
        return NVal(out, b.mult, False)

    # -- layernorm -----------------------------------------------------------
    def ln2(self, val):
        if val.unit:
            return NVal(val.t, _kappa(val.mult), True)
        key = id(val.t)
        if key in self.ln_cache:
            return self.ln_cache[key]
        nc = self.nc
        x = val.t
        x2 = self.acq([128, NFC, TOK], BF16)
        nc.vector.tensor_mul(x2[:, :, :], x[:, :, :], x[:, :, :])
        m_ps = self.ps_stat.tile([1, TOK], F32, tag=self.tag("st"))
        s_ps = self.ps_stat.tile([1, TOK], F32, tag=self.tag("st"))
        for kc in range(NFC):
            nc.tensor.matmul(m_ps[:, :], self.ones_col_bf[:, :], x[:, kc, :],
                             start=(kc == 0), stop=(kc == NFC - 1))
        for kc in range(NFC):
            nc.tensor.matmul(s_ps[:, :], self.ones_col_bf[:, :], x2[:, kc, :],
                             start=(kc == 0), stop=(kc == NFC - 1))
        self.rel_tile(x2)
        sm = self.acq([1, 3 * TOK], F32)
        mean, var, tmp = (sm[:, ts(i, TOK)] for i in range(3))
        nc.vector.tensor_scalar_mul(mean, m_ps[:, :], 1.0 / ISIZE)
        nc.vector.scalar_tensor_tensor(tmp, mean, -1.0, mean,
                                       op0=ALU.mult, op1=ALU.mult)
        nc.vector.scalar_tensor_tensor(var, s_ps[:, :], 1.0 / ISIZE, tmp,
                                       op0=ALU.mult, op1=ALU.add)
        epsp = EPS / (val.mult * val.mult)
        nc.scalar.activation(tmp, var, AF.Ln, bias=self.const_col(epsp, 1))
        rbf = self.acq([1, 2 * TOK], BF16)
        r_bf, mr_bf = rbf[:, ts(0, TOK)], rbf[:, ts(1, TOK)]
        nc.scalar.activation(r_bf, tmp, AF.Exp, scale=-0.5)
        nc.vector.tensor_mul(mr_bf, mean, r_bf)
        self.rel_tile(sm)
        rb_ps = self.ps_stat.tile([128, TOK], F32, tag=self.tag("st"))
        mrb_ps = self.ps_stat.tile([128, TOK], F32, tag=self.tag("st"))
        nc.tensor.matmul(rb_ps[:, :], self.ones_row_bf[:, :], r_bf,
                         start=True, stop=True)
        nc.tensor.matmul(mrb_ps[:, :], self.ones_row_bf[:, :], mr_bf,
                         start=True, stop=True)
        self.rel_tile(rbf)
        xh = self.acq([128, NFC, TOK], BF16)
        for fc in range(NFC):
            nc.vector.tensor_mul(xh[:, fc, :], x[:, fc, :], rb_ps[:, :])
            nc.vector.tensor_sub(xh[:, fc, :], xh[:, fc, :], mrb_ps[:, :])
        out = NVal(xh, 1.0, True)
        self.ln_cache[key] = out
        return out

    # -- final LN (token-major via transpose + bn_stats) + output DMA -------
    def final_out(self, val, out_hdl, out_g, out_beta):
        nc = self.nc
        epsp = EPS / (val.mult * val.mult)
        eps_col = self.const_col(epsp, 128)
        need_aff = not (np.all(out_g == 1.0) and not np.any(out_beta))
        if need_aff:
            gh = self.upload("og", np.tile(np.asarray(out_g, np.float32),
                                           (128, 1)), [128, ISIZE], F32)
            bh = self.upload("ob", np.tile(np.asarray(out_beta, np.float32),
                                           (128, 1)), [128, ISIZE], F32)
            gt = self.acq([128, ISIZE], F32)
            bt = self.acq([128, ISIZE], F32)
            nc.sync.dma_start(gt[:, :], gh[:, :])
            nc.sync.dma_start(bt[:, :], bh[:, :])
        x = val.t
        for tt in range(NTT):
            ps = self.ps_pool.tile([128, ISIZE], F32, tag=self.tag("ps"))
            for fc in range(NFC):
                nc.tensor.transpose(ps[:, ts(fc, 128)], x[:, fc, ts(tt, 128)],
                                    self.ident_bf)
            sm = self.acq([128, 9], F32)
            stats, mv, rstd = sm[:, 0:6], sm[:, 6:8], sm[:, 8:9]
            nc.vector.bn_stats(stats, ps[:, :])
            nc.vector.bn_aggr(mv, stats)
            nc.scalar.activation(rstd, mv[:, 1:2], AF.Ln, bias=eps_col)
            nc.scalar.activation(rstd, rstd, AF.Exp, scale=-0.5)
            ot = self.acq([128, ISIZE], F32)
            nc.vector.tensor_scalar(ot[:, :], ps[:, :], mv[:, 0:1], rstd,
                                    op0=ALU.subtract, op1=ALU.mult)
            if need_aff:
                nc.vector.tensor_mul(ot[:, :], ot[:, :], gt[:, :])
                nc.vector.tensor_add(ot[:, :], ot[:, :], bt[:, :])
            nc.sync.dma_start(out_hdl[ts(tt, 128), :], ot[:, :])
            self.rel_tile(sm)
            self.rel_tile(ot)


def _resolve_term(val, Wn):
    """val.actual @ Wn as (src_tile, W_abs, b_abs_or_None)."""
    Wn = np.asarray(Wn, np.float64)
    if val.lin is not None:
        src, A, sc, b = val.lin
        W = (A @ Wn) if A is not None else sc * Wn
        bb = (b @ Wn) if b is not None else None
        return (src, W, bb)
    return (val.t, val.mult * Wn, None)


def _emit_graph2(bld, np_in, routes):
    nc = bld.nc
    eW = np.asarray(np_in['edge_W'], np.float64)
    eb = np.asarray(np_in['edge_b'], np.float64)
    eg = np.asarray(np_in['edge_g'], np.float64)
    ebe = np.asarray(np_in['edge_beta'], np.float64)
    nW = np.asarray(np_in['node_W'], np.float64)
    nb = np.asarray(np_in['node_b'], np.float64)
    ng = np.asarray(np_in['node_g'], np.float64)
    nbe = np.asarray(np_in['node_beta'], np.float64)
    node_p = np.asarray(np_in['node_p'], np.float64)
    edge_p = np.asarray(np_in['edge_p'], np.float64)

    if any(r['act'] == 0 for r in routes):
        raise NotImplementedError("MHA: fall back to v1 emitter")

    # lifetimes
    last_use = {}
    used_src = set()
    for c, r in enumerate(routes):
        for sel in (r['q'], r['k'], r['v']):
            if sel is None:
                continue
            src = -2 if sel // 5 == 0 else r['snode'] + sel // 5
            used_src.add(src)
            last_use[src] = c

    outs = {}
    for nm, idx in (('inpute', -2), ('inputo', -1)):
        if idx in used_src:
            hdl = bld.upload(
                nm,
                [np.ascontiguousarray(
                    np.asarray(np_in[nm]).reshape(-1, ISIZE)
                    [i * TOK:(i + 1) * TOK].astype(ml_dtypes.bfloat16))
                 for i in range(NCORE)],
                [TOK, ISIZE], BF16)
            outs[idx] = bld.load_input2(hdl)

    # sources that feed LN'd edges want eager stats (flushed after the
    # next GEMM so they don't block the tensor queue)
    needs_ln = set()
    for r in routes:
        for sel in (r['q'], r['k'], r['v']):
            if sel is not None and sel % 5 <= 2:
                needs_ln.add(-2 if sel // 5 == 0 else r['snode'] + sel // 5)

    edge_cache = {}
    processed = set()

    def edge_value(r, c, sel, which):
        se, op = sel // 5, sel % 5
        inn = -2 if se == 0 else r['snode'] + se
        processed.add(inn)
        e = r['lind'] + se
        lind, nsrc = r['lind'], r['nsrc']
        ep = edge_p[:, lind:lind + nsrc, :].reshape(3, -1)
        logits = ep[{'q': 0, 'k': 1, 'v': 2}[which]]
        first5 = (which == 'v' and r['vmode'] == 'first5')
        if first5:
            logits = logits[:5]
        mask = _qmask(nsrc) if which == 'q' else r['km']
        if first5:
            mask = None
        s = _selw_np(logits, mask, sel)
        src = outs[inn]
        if op == 4:
            return NVal(src.t, s * src.mult, src.unit,
                        lin=(src.t, None, s * src.mult, None))
        if op == 3:
            key = ('p', e)
            if key not in edge_cache:
                edge_cache[key] = bld.mm2(
                    [[(src.t, src.mult * eW[e])]],
                    bias_np=eb[e] if np.any(eb[e]) else None)
            h = edge_cache[key]
            b_abs = s * eb[e] if np.any(eb[e]) else None
            return NVal(h.t, s, False,
                        lin=(src.t, s * src.mult * eW[e], None, b_abs))
        # ops 0/1/2: LN -> Linear (+relu/gelu)
        lnv = bld.ln2(src)
        wp = lnv.mult * (eg[e][:, None] * eW[e])
        bp = ebe[e] @ (eg[e][:, None] * eW[e]) + eb[e]
        have_b = np.any(bp)
        # which ops does this node apply to edge e?
        ops_here = set()
        for sl in (r['q'], r['k'], r['v']):
            if sl is not None and sl // 5 == se:
                ops_here.add(sl % 5)
        if ops_here == {0}:
            key = ('r', e)
            if key not in edge_cache:
                edge_cache[key] = bld.mm2(
                    [[(lnv.t, wp)]], bias_np=bp if have_b else None,
                    epi="relu")
            return NVal(edge_cache[key].t, s, False)
        key = ('h', e)
        if key not in edge_cache:
            edge_cache[key] = bld.mm2(
                [[(lnv.t, wp)]], bias_np=bp if have_b else None)
        h = edge_cache[key]
        if op == 2:
            b_abs = s * bp if have_b else None
            return NVal(h.t, s, False, lin=(lnv.t, s * wp, None, b_abs))
        fkey = ('relu' if op == 0 else 'gelu', e)
        if fkey not in edge_cache:
            edge_cache[fkey] = (bld.relu_of(h) if op == 0
                               else bld.gelu_of(h))
        return NVal(edge_cache[fkey].t, s * edge_cache[fkey].mult, False)

    def affine2(lnval, c, aw):
        g, bta = ng[c], nbe[c]
        if np.all(g == 1.0) and not np.any(bta):
            return NVal(lnval.t, lnval.mult * aw, lnval.unit)
        sc = bld.upload_bias(aw * lnval.mult * g)
        bi = bld.upload_bias(aw * bta)
        out = bld.acq([128, NFC, TOK], BF16)
        for fc in range(NFC):
            nc.scalar.activation(out[:, fc, :], lnval.t[:, fc, :],
                                 AF.Identity, scale=sc[:, fc:fc + 1],
                                 bias=bi[:, fc:fc + 1])
        return NVal(out, 1.0, False)

    def grouped_terms(pairs):
        """pairs: list of (val, Wn). Resolve lin provenance, merge terms
        sharing a source tile, order groups by tile creation time."""
        agg = {}
        for val, Wn in pairs:
            t, W, b = _resolve_term(val, Wn)
            key = id(t)
            if key in agg:
                agg[key][1] += W
                if b is not None:
                    agg[key][2] = b if agg[key][2] is None else agg[key][2] + b
            else:
                agg[key] = [t, W.copy(), b]
        glist = sorted(agg.values(), key=lambda g: bld.order.get(id(g[0]), 0))
        groups = [[(g[0], g[1])] for g in glist]
        btot = None
        for g in glist:
            if g[2] is not None:
                btot = g[2] if btot is None else btot + g[2]
        return groups, btot

    for c, r in enumerate(routes):
        act = r['act']
        aw = float(_softmax_np(node_p[c] / TAU)[act])
        qv = edge_value(r, c, r['q'], 'q')
        kv = edge_value(r, c, r['k'], 'k') if r['k'] is not None else None
        vv = edge_value(r, c, r['v'], 'v') if r['v'] is not None else None

        if act == 1:
            gg, gb = grouped_terms([(qv, nW[c, 0])])
            bb = nb[c, 0] + (gb if gb is not None else 0)
            g = bld.mm2(gg, bias_np=bb if np.any(bb) else None)
            g = bld.gelu_of(g)
            kg, kb = grouped_terms([(kv, nW[c, 1])])
            bb = nb[c, 1] + (kb if kb is not None else 0)
            kk = bld.mm2(kg, bias_np=bb if np.any(bb) else None)
            p = bld.acq([128, NFC, TOK], BF16)
            nc.vector.tensor_mul(p[:, :, :], g.t[:, :, :], kk.t[:, :, :])
            pv = NVal(p, g.mult * kk.mult, False)
            assert not np.any(nb[c, 3])
            outs[c] = bld.mm2([[(pv.t, aw * pv.mult * nW[c, 3])]],
                              resid=(qv, aw))
        elif act == 2:
            s2 = bld.axpy2(bld.axpy2(qv, kv), vv)
            outs[c] = affine2(bld.ln2(s2), c, aw)
        elif act == 3:
            groups, btot = grouped_terms(
                [(qv, nW[c, 0]), (kv, nW[c, 1]), (vv, nW[c, 2])])
            inner = bld.mm2(groups, bias_np=btot, epi="relu")
            assert not np.any(nb[c, 3])
            outs[c] = bld.mm2([[(inner.t, aw * nW[c, 3])]], resid=(qv, aw))
        elif act == 4:
            sig = bld.sigmoid_of(kv)
            prod = bld.acq([128, NFC, TOK], BF16)
            nc.vector.tensor_mul(prod[:, :, :], qv.t[:, :, :], sig[:, :, :])
            bld.rel_tile(sig)
            outs[c] = bld.axpy2(NVal(prod, qv.mult * aw, False),
                                NVal(vv.t, vv.mult * aw, vv.unit))
        elif act == 5:
            kg, kb = grouped_terms([(kv, nW[c, 1])])
            bb = nb[c, 1] + (kb if kb is not None else 0)
            kk = bld.mm2(kg, bias_np=bb if np.any(bb) else None)
            kk = bld.gelu_of(kk)
            outs[c] = bld.axpy2(NVal(kk.t, kk.mult * aw, False),
                                NVal(qv.t, qv.mult * aw, qv.unit))
        elif act == 6:
            outs[c] = bld.axpy2(NVal(qv.t, qv.mult * aw, qv.unit),
                                NVal(kv.t, kv.mult * aw, kv.unit))
        else:  # act 7
            outs[c] = affine2(bld.ln2(qv), c, aw)

        if c in needs_ln and not outs[c].unit:
            bld.pending_ln.append(outs[c])

        # ---- lifetime sweep ----
        for s_idx in [s for s, lc in last_use.items() if lc == c]:
            outs.pop(s_idx, None)
        keep = set()
        for v in outs.values():
            keep.add(id(v.t))
        for k in [k for k in bld.ln_cache
                  if k not in keep and bld.ln_cache[k].t is not None]:
            pass  # prune below
        live_src = {id(v.t) for v in outs.values()}
        for k in list(bld.ln_cache):
            if k not in live_src:
                lv = bld.ln_cache.pop(k)
                if id(lv.t) not in live_src:
                    bld.rel_tile(lv.t)
        keep = {id(v.t) for v in outs.values()}
        keep |= {id(lv.t) for lv in bld.ln_cache.values()}
        keep |= {id(v.t) for v in bld.pending_ln}
        for t in bld.window:
            if id(t) not in keep:
                bld.rel_tile(t)
        bld.window = [t for t in bld.window if id(t) in keep]
        edge_cache.clear()

    rem = [outs[i] for i in range(NNOD) if i not in processed]
    acc = rem[0]
    for t in rem[1:]:
        acc = bld.axpy2(acc, t)
    return acc


# ---------------------------------------------------------------------------
# Graph emission
# ---------------------------------------------------------------------------

def _emit_graph(bld, np_in, routes, core_mask_bias):
    nc = bld.nc
    eW = np.asarray(np_in['edge_W'], np.float64)
    eb = np.asarray(np_in['edge_b'], np.float64)
    eg = np.asarray(np_in['edge_g'], np.float64)
    ebe = np.asarray(np_in['edge_beta'], np.float64)
    nW = np.asarray(np_in['node_W'], np.float64)
    nb = np.asarray(np_in['node_b'], np.float64)
    ng = np.asarray(np_in['node_g'], np.float64)
    nbe = np.asarray(np_in['node_beta'], np.float64)
    node_p = np.asarray(np_in['node_p'], np.float64)
    edge_p = np.asarray(np_in['edge_p'], np.float64)

    # source lifetimes
    last_use = {}
    use_nodes = {}
    used_src = set()
    for c, r in enumerate(routes):
        for sel in (r['q'], r['k'], r['v']):
            if sel is None:
                continue
            se = sel // 5
            src = -2 if se == 0 else r['snode'] + se
            used_src.add(src)
            last_use[src] = c
            use_nodes.setdefault(src, []).append(c)
    for i in range(NNOD):
        if i not in use_nodes:
            use_nodes[i] = [NNOD]  # survives to the final sum

    # sources that later feed an LN'd edge (ops 0/1/2) want their LN
    # statistics computed as soon as they exist, so fused-LN consumers
    # never stall on the stats chain.
    needs_stats = set()
    for r in routes:
        for sel in (r['q'], r['k'], r['v']):
            if sel is None:
                continue
            se, op = sel // 5, sel % 5
            if op <= 2:
                needs_stats.add(-2 if se == 0 else r['snode'] + se)

    outs = {}
    for nm, idx in (('inpute', -2), ('inputo', -1)):
        if idx in used_src:
            hdl = bld.upload(
                nm,
                [np.ascontiguousarray(
                    np.asarray(np_in[nm]).reshape(-1, ISIZE)
                    [i * TOK:(i + 1) * TOK].astype(ml_dtypes.bfloat16))
                 for i in range(NCORE)],
                [TOK, ISIZE], BF16)
            outs[idx] = bld.load_input_fm(hdl)
            if idx in needs_stats:
                bld.ln_stats(outs[idx])

    edge_cache = {}
    processed = set()

    def edge_value(r, sel, which):
        se, op = sel // 5, sel % 5
        inn = -2 if se == 0 else r['snode'] + se
        processed.add(inn)
        e = r['lind'] + se
        lind, nsrc = r['lind'], r['nsrc']
        ep = edge_p[:, lind:lind + nsrc, :].reshape(3, -1)
        logits = ep[{'q': 0, 'k': 1, 'v': 2}[which]]
        first5 = (which == 'v' and r['vmode'] == 'first5')
        if first5:
            logits = logits[:5]
        mask = _qmask(nsrc) if which == 'q' else r['km']
        if first5:
            mask = None
        s = _selw_np(logits, mask, sel)
        src = outs[inn]
        if op == 4:
            return Val(src.dt, src.mult * s, src.unit)
        if op == 3:
            key = ('p', e)
            if key not in edge_cache:
                edge_cache[key] = bld.matmul_fm(
                    [(src, eW[e])],
                    bias_np=eb[e] if np.any(eb[e]) else None,
                    out_f32=False, out_bf=True)
            return Val(edge_cache[key].dt, s, False)
        key = ('h', e)
        if key not in edge_cache:
            wp = eg[e][:, None] * eW[e]
            bp = ebe[e] @ eW[e] + eb[e]
            if src.unit:
                lnv = bld.ln_fm(src)
                edge_cache[key] = bld.matmul_fm(
                    [(lnv, wp)], bias_np=bp if np.any(bp) else None,
                    out_f32=False, out_bf=True)
            else:
                edge_cache[key] = bld.matmul_fm_ln(
                    src, wp, bias_np=bp if np.any(bp) else None,
                    out_f32=False, out_bf=True)
        h = edge_cache[key]
        if op == 2:
            return Val(h.dt, s, False)
        fkey = ('relu' if op == 0 else 'gelu', e)
        if fkey not in edge_cache:
            edge_cache[fkey] = bld.act_pass(
                h, AF.Relu if op == 0 else AF.Gelu_apprx_tanh)
        return Val(edge_cache[fkey].dt, s, False)

    def affine_node(ln_val, c, aw):
        g, bta = ng[c], nbe[c]
        if np.all(g == 1.0) and not np.any(bta):
            return Val(ln_val.dt, ln_val.mult * aw, True)
        sc = bld.upload_bias(aw * ln_val.mult * g)
        bi = bld.upload_bias(aw * bta)
        dt = DT(bld)
        dt.bf = bld.acq([128, NFC, TOK], BF16)
        src = ln_val.dt.any()
        for fc in range(NFC):
            nc.scalar.activation(dt.bf[:, fc, :], src[:, fc, :], AF.Identity,
                                 scale=sc[:, fc:fc + 1], bias=bi[:, fc:fc + 1])
        return Val(dt, 1.0, False)

    def reachable_ids():
        s = set()
        vals = list(outs.values()) + list(edge_cache.values()) + \
            [lv for _, lv in bld.ln_cache.values()]
        for v in vals:
            for t in v.dt.tiles():
                s.add(id(t))
        for _, m_bf, rb_sb in bld.stats_cache.values():
            s.add(id(m_bf))
            s.add(id(rb_sb))
        return s

    bld.live_provider = reachable_ids
    flush = bld.flush

    for c, r in enumerate(routes):
        act = r['act']
        aw = float(_softmax_np(node_p[c] / TAU)[act])
        qv = edge_value(r, r['q'], 'q')
        flush([qv])
        kv = edge_value(r, r['k'], 'k') if r['k'] is not None else None
        flush([qv, kv])
        vv = edge_value(r, r['v'], 'v') if r['v'] is not None else None
        flush([qv, kv, vv])

        if act == 0:
            mask_nm = 'tgt_pad_mask' if r['ktype'] == -1 else 'src_pad_mask'
            outs[c] = bld.emit_mha(
                qv, kv, vv, nW[c], nb[c], ng[c], nbe[c], aw,
                core_mask_bias(np.asarray(np_in[mask_nm])))
        elif act == 1:
            g = bld.matmul_fm([(qv, nW[c, 0])],
                              bias_np=nb[c, 0] if np.any(nb[c, 0]) else None,
                              epi="gelu", out_f32=False, out_bf=True)
            kk = bld.matmul_fm([(kv, nW[c, 1])],
                               bias_np=nb[c, 1] if np.any(nb[c, 1]) else None,
                               out_f32=False, out_bf=True)
            p = bld.mul_vals(g, kk)
            if np.any(nb[c, 3]):
                pr = bld.matmul_fm([(p, aw * nW[c, 3])], bias_np=aw * nb[c, 3],
                                   out_f32=True)
                outs[c] = bld.axpy(Val(qv.dt, qv.mult * aw, False),
                                   Val(pr.dt, 1.0, False))
            else:
                ps = bld.mm_psums([(p, aw * nW[c, 3])])
                outs[c] = bld.add_psum_resid(qv, aw * qv.mult, ps)
        elif act == 2:
            s2 = bld.axpy(bld.axpy(qv, kv, out_bf=True), vv, out_bf=True)
            ln = bld.ln_fm(s2, out_f32=False, out_bf=True)
            outs[c] = affine_node(ln, c, aw)
        elif act == 3:
            inner = bld.matmul_fm([(qv, nW[c, 0]), (kv, nW[c, 1]),
                                   (vv, nW[c, 2])], epi="relu",
                                  out_f32=False, out_bf=True)
            if np.any(nb[c, 3]):
                pr = bld.matmul_fm([(inner, aw * nW[c, 3])],
                                   bias_np=aw * nb[c, 3], out_f32=True)
                outs[c] = bld.axpy(Val(qv.dt, qv.mult * aw, False),
                                   Val(pr.dt, 1.0, False))
            else:
                ps = bld.mm_psums([(inner, aw * nW[c, 3])])
                outs[c] = bld.add_psum_resid(qv, aw * qv.mult, ps)
        elif act == 4:
            sg = bld.act_pass(kv, AF.Sigmoid)
            p = bld.mul_vals(qv, sg)
            outs[c] = bld.axpy(Val(p.dt, p.mult * aw, False),
                               Val(vv.dt, vv.mult * aw, vv.unit))
        elif act == 5:
            kk = bld.matmul_fm([(kv, nW[c, 1])],
                               bias_np=nb[c, 1] if np.any(nb[c, 1]) else None,
                               epi="gelu", out_f32=False, out_bf=True)
            outs[c] = bld.axpy(Val(kk.dt, aw, False),
                               Val(qv.dt, qv.mult * aw, qv.unit))
        elif act == 6:
            outs[c] = bld.axpy(Val(qv.dt, qv.mult * aw, qv.unit),
                               Val(kv.dt, kv.mult * aw, kv.unit))
        else:
            ln = bld.ln_fm(qv, out_f32=False, out_bf=True)
            outs[c] = affine_node(ln, c, aw)

        if c in needs_stats and not outs[c].unit:
            bld.ln_stats(outs[c])

        # ---- lifetime bookkeeping ----
        dead_tiles = []
        for s_idx in [s for s, lc in last_use.items() if lc == c]:
            v = outs.pop(s_idx, None)
            if v is not None:
                dead_tiles += v.dt.tiles()
        # prune LN/stats cache entries whose source is no longer alive
        alive_dts = {id(v.dt) for v in outs.values()}
        for key in [k for k in bld.ln_cache if k not in alive_dts]:
            _, lv = bld.ln_cache.pop(key)
            dead_tiles += lv.dt.tiles()
        for key in [k for k in bld.stats_cache if k[0] not in alive_dts]:
            _, m_bf, rb_sb = bld.stats_cache.pop(key)
            dead_tiles += [m_bf, rb_sb]
        edge_cache.clear()
        keep = reachable_ids()
        for t in bld.window + dead_tiles:
            if id(t) not in keep:
                bld.rel_tile(t)
        bld.window = []
        # spill node outputs whose next use is at least two nodes away
        for s_idx, v in list(outs.items()):
            nxt = min((u for u in use_nodes.get(s_idx, [NNOD]) if u > c),
                      default=NNOD)
            if nxt > c + 1:
                lv = bld.ln_cache.get(id(v.dt))
                if lv is not None:
                    lv[1].dt.do_spill()
                v.dt.do_spill()

    rem = [outs[i] for i in range(NNOD) if i not in processed]
    acc = rem[0]
    for t in rem[1:]:
        acc = bld.axpy(acc, t)
    return acc


def _emit_final(bld, acc, out_hdl, out_g, out_beta):
    """Transpose to token-major, final LNraw (+ optional affine), DMA out."""
    nc = bld.nc
    x = acc.dt.need_f32()
    epsp = EPS / (acc.mult * acc.mult)
    need_aff = not (np.all(out_g == 1.0) and not np.any(out_beta))
    if need_aff:
        gh = bld.upload("og", np.tile(np.asarray(out_g, np.float32),
                                      (128, 1)), [128, ISIZE], F32)
        bh = bld.upload("ob", np.tile(np.asarray(out_beta, np.float32),
                                      (128, 1)), [128, ISIZE], F32)
        gt = bld.acq([128, ISIZE], F32)
        bt = bld.acq([128, ISIZE], F32)
        nc.sync.dma_start(gt[:, :], gh[:, :])
        nc.sync.dma_start(bt[:, :], bh[:, :])
    eps_col = bld.const_col(epsp, 128)
    for tt in range(NTT):
        ps = bld.ps_pool.tile([128, ISIZE], F32, tag="ps")
        for fc in range(NFC):
            nc.tensor.transpose(ps[:, ts(fc, 128)], x[:, fc, ts(tt, 128)],
                                bld.ident_f32)
        sm = bld.acq([128, 12], F32)
        stats, mv, rstd = sm[:, 0:6], sm[:, 6:8], sm[:, 8:9]
        nc.vector.bn_stats(stats, ps[:, :])
        nc.vector.bn_aggr(mv, stats)
        nc.scalar.activation(rstd, mv[:, 1:2], AF.Ln, bias=eps_col)
        nc.scalar.activation(rstd, rstd, AF.Exp, scale=-0.5)
        ot = bld.acq([128, ISIZE], F32)
        nc.vector.tensor_scalar(ot[:, :], ps[:, :], mv[:, 0:1], rstd,
                                op0=ALU.subtract, op1=ALU.mult)
        if need_aff:
            nc.vector.tensor_mul(ot[:, :], ot[:, :], gt[:, :])
            nc.vector.tensor_add(ot[:, :], ot[:, :], bt[:, :])
        nc.sync.dma_start(out_hdl[ts(tt, 128), :], ot[:, :])
        bld.rel_tile(sm)
        bld.rel_tile(ot)


def _build_and_run(inputs, trace=False, **run_kwargs):
    np_in = {k: np.asarray(v) for k, v in inputs.items()}
    routes = _routing(np_in['node_p'], np_in['edge_p'])

    def core_mask_bias(mask_np):
        if not np.any(mask_np):
            return None
        arrs = []
        for core in range(NCORE):
            vec = np.asarray(mask_np[core // 2, 0, :], bool)
            mb = np.zeros((128, 2 * NTT), np.float32)
            for kc8 in range(2 * NTT):
                base = (kc8 // NTT) * TOK + (kc8 % NTT) * 128
                mb[:, kc8] = np.where(vec[base:base + 128], -1e9, 0.0)
            arrs.append(mb)
        return arrs

    nc = None
    uploads = None
    try:
        nc = bass.Bass(num_devices=NCORE)
        out_hdl = nc.declare_dram_parameter("out", [TOK, ISIZE], F32,
                                            isOutput=True)
        with FixedTileContext(nc) as tc:
            with ExitStack() as ctx:
                bld = B2(nc, tc, ctx)
                acc = _emit_graph2(bld, np_in, routes)
                bld.final_out(acc, out_hdl, np.asarray(np_in['out_g']),
                              np.asarray(np_in['out_beta']))
                uploads = bld.uploads
    except Exception:
        import traceback
        traceback.print_exc()
        nc = None
    if nc is None:  # fall back to the proven v1 emitter
        nc = bass.Bass(num_devices=NCORE)
        out_hdl = nc.declare_dram_parameter("out", [TOK, ISIZE], F32,
                                            isOutput=True)
        with FixedTileContext(nc) as tc:
            with ExitStack() as ctx:
                bld = Builder(nc, tc, ctx)
                acc = _emit_graph(bld, np_in, routes, core_mask_bias)
                _emit_final(bld, acc, out_hdl, np.asarray(np_in['out_g']),
                            np.asarray(np_in['out_beta']))
                uploads = bld.uploads
    _hoist_excess_waits(nc)
    in_maps = [{nm: arrs[i] for nm, arrs in uploads.items()}
               for i in range(NCORE)]
    res = run_bass_kernel_spmd(nc, in_maps, core_ids=list(range(NCORE)),
                               trace=trace, **run_kwargs)
    out = np.concatenate([res.results[i]['out'] for i in range(NCORE)], 0)
    return out.reshape(B, SLEN, ISIZE).astype(np.float32), res


def kernel(**inputs):
    out, _ = _build_and_run(inputs)
    return out

